# revision 9
# baseline (speedup 1.0000x reference)
"""Trainium2 Bass kernel for nn_Attention_54580444397738 (gnn_message_passing).

Math per batch b (B=8, N=128, H=256, C=16):
  proj         = local @ W_apair                                     [N, H]
  pre[i,j,:]   = proj[i,:] + proj[j,:] + binary[i,j,:] @ W_binary
                 + b_apair + b_binary                                [N, N, H]
  score[i,j]   = sigmoid(relu(pre[i,j,:]) . W_att + b_att)           [N, N]
  glob         = score @ local                                       [N, H]
  local_pair [i,j,:] = local[i,:] + local[j,:]                       (output 1)
  global_pair[i,j,:] = glob[i,:]  + glob[j,:]                        (output 2)

Key algebraic simplification: einsum("bijh,hk->bijk", local_pair, W_apair)
= proj[i,:] + proj[j,:], so the N^2xHxH matmul collapses to an NxHxH one.

Sharding: data-parallel over batch B across the 8 cores (1 batch per core).
The two outputs (2 x 16 MB fp32 per core) dominate -> memory-bound; all
compute is hidden under the ~32 MB/core of output DMA.

Per-core device plan (all j-on-partitions layout, i = row-block loop):
  - binary is DMA-loaded as binp[j, (i, c_pad32)] with a ones-column at c=16;
    PE-transposing 128-wide chunks yields lhsT blocks [17, 128] at 32-aligned
    partitions: one K=17 matmul then computes binary@W_binary + bias
    (bias rides rhs row 16 of a 4x-replicated W_binary tile).
  - pre for a pair (i, i+1) accumulates in one PSUM bank [128, 512]:
    ones@projFlat-row (proj[i,:]) + I@projW (proj[j,:]) + binT@Wx4.
  - ACT relu -> SBUF, then DVE tensor_tensor_reduce against a broadcast
    W_att tile produces logits columns (fused dot product, init = b_att).
  - scoreT[j,i] = sigmoid(logits); glob = scoreT.T @ local on PE.
  - Output tiles [j=128, h=256] per i are built by PE broadcast-matmuls into
    PSUM and drained by ACT/DVE (variants A/B/C to balance engines), staged
    8 tiles at a time in SBUF and stored with 1 MB DMAs.
"""

import numpy as np

B, N, H, BIN = 8, 128, 256, 16
NCORES = 8
CPAD = 32        # c dim padded 16 -> 32 so transposed blocks land 32-aligned
IG = 4           # i's per binary-transpose group (4 * 32 = 128)
STAGE_I = 8      # output tiles per staged 1MB DMA store
PROLOG = 16      # local_pair tiles emitted before attention work starts

# Per-tile output production variants (cycled):
#  'A': PE 2 matmuls -> full tile in PSUM, ACT copies to SBUF
#  'B': PE 2 matmuls -> full tile in PSUM, DVE copies to SBUF
#  'C': PE 1 matmul (row broadcast) -> PSUM, DVE adds tile+row to SBUF
VAR_LP = "AACACAAC"   # phase 1 (local_pair): keep PE/DVE light, ACT does copies
VAR_GP = "AABCAABC"   # phase 2 (global_pair): DMA-bound, spread evenly

_cache = {}


def _body(tc, io, reps=1):
    import concourse.bass as bass
    import concourse.mybir as mybir
    from concourse.masks import make_identity
    from contextlib import ExitStack, nullcontext

    nc = tc.nc
    ts = bass.ts
    f32 = mybir.dt.float32
    Relu = mybir.ActivationFunctionType.Relu
    Sigmoid = mybir.ActivationFunctionType.Sigmoid
    mult = mybir.AluOpType.mult
    add = mybir.AluOpType.add

    local_d, binary_d, wap_d, bap_d, wbin_d, bbin_d, watt_d, batt_d, lp_d, gp_d = io

    ctx = ExitStack()
    with ctx:
        persist = ctx.enter_context(tc.tile_pool(name="persist", bufs=1))
        binTp = ctx.enter_context(tc.tile_pool(name="binTp", bufs=6))
        att2p = ctx.enter_context(tc.tile_pool(name="att2p", bufs=3))
        stagep = ctx.enter_context(tc.tile_pool(name="stagep", bufs=3))
        prep = ctx.enter_context(tc.tile_pool(name="prep", bufs=2, space="PSUM"))
        outpp = ctx.enter_context(tc.tile_pool(name="outpp", bufs=4, space="PSUM"))
        dramp = ctx.enter_context(tc.tile_pool(name="dramp", bufs=1, space="DRAM"))

        # timing builds wrap the whole body in a device-side loop
        loop = tc.For_i(0, reps, 1) if reps > 1 else nullcontext()
        ctx.enter_context(loop)

        # ---------------- persistent setup ----------------
        identity = persist.tile([128, 128], f32, tag="identity")
        make_identity(nc, identity)
        onesT = persist.tile([128, 128], f32, tag="onesT")
        nc.gpsimd.memset(onesT, 1.0)

        localSb = persist.tile([N, H], f32, tag="localSb")
        nc.sync.dma_start(out=localSb, in_=local_d)

        # flat row views for broadcast-row matmuls; rows live at partitions
        # {0,32,64,96} so lhsT/rhs base partitions stay 32-aligned.
        def load_flat(tag, src_flat4):
            t = persist.tile([97, 32 * H], f32, tag=tag)
            for q in range(4):
                nc.sync.dma_start(out=t[32 * q : 32 * q + 1, :], in_=src_flat4[q : q + 1])
            return t

        localFlat = load_flat("localFlat", local_d.rearrange("(a x) h -> a (x h)", a=4))

        wap0 = persist.tile([128, H], f32, tag="wap0")
        nc.sync.dma_start(out=wap0, in_=wap_d[0:128])
        wap1 = persist.tile([128, H], f32, tag="wap1")
        nc.sync.dma_start(out=wap1, in_=wap_d[128:256])

        biasA = persist.tile([1, H], f32, tag="biasA")
        nc.sync.dma_start(out=biasA, in_=bap_d.unsqueeze(0))
        biasB = persist.tile([1, H], f32, tag="biasB")
        nc.sync.dma_start(out=biasB, in_=bbin_d.unsqueeze(0))
        biasRow = persist.tile([1, H], f32, tag="biasRow")
        nc.vector.tensor_add(out=biasRow, in0=biasA, in1=biasB)

        wattRow = persist.tile([1, H], f32, tag="wattRow")
        nc.sync.dma_start(out=wattRow, in_=watt_d.rearrange("k o -> o k"))
        battRow = persist.tile([1, 1], f32, tag="battRow")
        nc.sync.dma_start(out=battRow, in_=batt_d.unsqueeze(0))

        # Wx4: W_binary replicated at partitions {0,32,64,96}; row 32m+16 = bias
        wx4 = persist.tile([128, H], f32, tag="wx4")
        nc.gpsimd.memset(wx4, 0.0)
        for m in range(4):
            nc.sync.dma_start(out=wx4[32 * m : 32 * m + 16, :], in_=wbin_d)
            nc.sync.dma_start(out=wx4[32 * m + 16 : 32 * m + 17, :], in_=biasRow)

        # broadcast W_att across partitions; b_att as a [128,1] column
        pw = outpp.tile([128, H], f32, tag="outp")
        nc.tensor.matmul(pw, lhsT=onesT[0:1, :], rhs=wattRow, start=True, stop=True)
        wattB = persist.tile([128, H], f32, tag="wattB")
        nc.vector.tensor_copy(out=wattB, in_=pw)
        pb = outpp.tile([128, H], f32, tag="outp")
        nc.tensor.matmul(pb[:, 0:1], lhsT=onesT[0:1, :], rhs=battRow, start=True, stop=True)
        battCol = persist.tile([128, 1], f32, tag="battCol")
        nc.vector.tensor_copy(out=battCol, in_=pb[:, 0:1])

        # localT = local^T  (h on partitions), then projW = local @ W_apair
        localT = persist.tile([128, H], f32, tag="localT")
        for hb in range(2):
            tp = outpp.tile([128, H], f32, tag="outp")
            nc.tensor.transpose(tp[:, 0:128], localSb[:, ts(hb, 128)], identity)
            nc.vector.tensor_copy(out=localT[:, ts(hb, 128)], in_=tp[:, 0:128])
        pp = outpp.tile([128, H], f32, tag="outp")
        nc.tensor.matmul(pp, lhsT=localT[:, 0:128], rhs=wap0, start=True, stop=False)
        nc.tensor.matmul(pp, lhsT=localT[:, 128:256], rhs=wap1, start=False, stop=True)
        projW = persist.tile([128, H], f32, tag="projW")
        nc.vector.tensor_copy(out=projW, in_=pp)

        projDram = dramp.tile([N, H], f32, tag="projDram")
        nc.sync.dma_start(out=projDram, in_=projW)
        projFlat = load_flat("projFlat", projDram.rearrange("(a x) h -> a (x h)", a=4))

        # binp[j, (i, c32)]: c 0..15 = binary[., i, j, .], c16 = 1.0 (bias lane)
        binp = persist.tile([128, N * CPAD], f32, tag="binp")
        nc.gpsimd.memset(binp, 0.0)
        binp3 = binp.rearrange("p (i c) -> p i c", c=CPAD)
        nc.gpsimd.memset(binp3[:, :, 16:17], 1.0)
        for q in range(4):
            nc.sync.dma_start(
                out=binp3[:, ts(q, 32), 0:BIN],
                in_=binary_d[ts(q, 32)].rearrange("i j c -> j i c"),
            )

        logits = persist.tile([128, N], f32, tag="logits")
        ttrS = persist.tile([128, H], f32, tag="ttrS")
        binT = {}

        # ---------------- helpers ----------------
        def row_rhs(xFlat, i, width):
            q, r = divmod(i, 32)
            return xFlat[32 * q : 32 * q + 1, r * H : r * H + width]

        def row_lhsT(i):
            q = i // 32
            return onesT[32 * q : 32 * q + 1, :]

        def row_tp(i):
            return (32 * (i // 32), 0)

        def out_tile(i, xSb, xFlat, stage, variant):
            dst = stage[:, ts(i % STAGE_I, H)]
            po = outpp.tile([128, H], f32, tag="outp")
            if variant in ("A", "B"):
                nc.tensor.matmul(po, lhsT=identity, rhs=xSb, start=True, stop=False)
                nc.tensor.matmul(po, lhsT=row_lhsT(i), rhs=row_rhs(xFlat, i, H),
                                 start=False, stop=True, tile_position=row_tp(i))
                if variant == "A":
                    nc.scalar.copy(out=dst, in_=po)
                else:
                    nc.vector.tensor_copy(out=dst, in_=po)
            else:  # 'C'
                nc.tensor.matmul(po, lhsT=row_lhsT(i), rhs=row_rhs(xFlat, i, H),
                                 start=True, stop=True, tile_position=row_tp(i))
                nc.vector.tensor_add(out=dst, in0=xSb, in1=po)

        def flush_stage(i0, stage, dram_out):
            nc.sync.dma_start(
                out=dram_out[i0 : i0 + STAGE_I].rearrange("i j h -> j i h"),
                in_=stage.rearrange("p (i h) -> p i h", h=H),
            )

        def attn_step(i):
            g, il = divmod(i, IG)
            if il == 0:  # transpose this binary group: [j,(i4,c32)] -> [(i4,c32),j]
                tp = outpp.tile([128, H], f32, tag="outp")
                nc.tensor.transpose(tp[:, 0:128], binp[:, ts(g, 128)], identity)
                bt = binTp.tile([128, 128], f32, tag="binT")
                nc.vector.tensor_copy(out=bt, in_=tp[:, 0:128])
                binT[g] = bt
            if i % 2 == 1:
                return
            pre = prep.tile([128, 2 * H], f32, tag="pre")
            nc.tensor.matmul(pre, lhsT=row_lhsT(i), rhs=row_rhs(projFlat, i, 2 * H),
                             start=True, stop=False, tile_position=row_tp(i))
            for m in range(2):
                ii = i + m
                gg, iil = divmod(ii, IG)
                nc.tensor.matmul(pre[:, ts(m, H)], lhsT=identity, rhs=projW,
                                 start=False, stop=False)
                nc.tensor.matmul(
                    pre[:, ts(m, H)],
                    lhsT=binT[gg][32 * iil : 32 * iil + 17, :],
                    rhs=wx4[32 * iil : 32 * iil + 17, :],
                    start=False, stop=(m == 1), tile_position=(32 * iil, 0),
                )
            a2 = att2p.tile([128, 2 * H], f32, tag="att2")
            nc.scalar.activation(out=a2, in_=pre, func=Relu)
            for m in range(2):
                nc.vector.affine_mul_reduce(
                    out=ttrS, accum_out=logits[:, i + m : i + m + 1],
                    in0=a2[:, ts(m, H)], in1=wattB, scale=1.0, bias=0.0,
                )

        # ---------------- phase 1: local_pair + attention ----------------
        stage = None
        for i in range(N):
            if i % STAGE_I == 0:
                stage = stagep.tile([128, STAGE_I * H], f32, tag="stage")
            out_tile(i, localSb, localFlat, stage, VAR_LP[i % len(VAR_LP)])
            if i % STAGE_I == STAGE_I - 1:
                flush_stage(i - STAGE_I + 1, stage, lp_d)
            if i >= PROLOG:
                attn_step(i - PROLOG)
        for i in range(N - PROLOG, N):
            attn_step(i)

        # ---------------- scores -> glob ----------------
        scoreT = persist.tile([128, N], f32, tag="scoreT")
        nc.scalar.activation(out=scoreT, in_=logits, func=Sigmoid, bias=battCol)
        pg = outpp.tile([128, H], f32, tag="outp")
        nc.tensor.matmul(pg, lhsT=scoreT, rhs=localSb, start=True, stop=True)
        globSb = persist.tile([128, H], f32, tag="globSb")
        nc.vector.tensor_copy(out=globSb, in_=pg)
        globDram = dramp.tile([N, H], f32, tag="globDram")
        nc.sync.dma_start(out=globDram, in_=globSb)
        globFlat = load_flat("globFlat", globDram.rearrange("(a x) h -> a (x h)", a=4))

        # ---------------- phase 2: global_pair ----------------
        for i in range(N):
            if i % STAGE_I == 0:
                stage = stagep.tile([128, STAGE_I * H], f32, tag="stage")
            out_tile(i, globSb, globFlat, stage, VAR_GP[i % len(VAR_GP)])
            if i % STAGE_I == STAGE_I - 1:
                flush_stage(i - STAGE_I + 1, stage, gp_d)


def _build(reps=1):
    import concourse.bass as bass  # noqa: F401
    from concourse import bacc
    import concourse.mybir as mybir
    import concourse.tile as tile

    f32 = mybir.dt.float32
    nc = bacc.Bacc(
        "TRN2",
        target_bir_lowering=False,
        debug=False,
        enable_asserts=False,
        num_devices=NCORES,
    )
    io = (
        nc.dram_tensor("local", [N, H], f32, kind="ExternalInput").ap(),
        nc.dram_tensor("binary", [N, N, BIN], f32, kind="ExternalInput").ap(),
        nc.dram_tensor("w_apair", [H, H], f32, kind="ExternalInput").ap(),
        nc.dram_tensor("b_apair", [H], f32, kind="ExternalInput").ap(),
        nc.dram_tensor("w_binary", [BIN, H], f32, kind="ExternalInput").ap(),
        nc.dram_tensor("b_binary", [H], f32, kind="ExternalInput").ap(),
        nc.dram_tensor("w_att", [H, 1], f32, kind="ExternalInput").ap(),
        nc.dram_tensor("b_att", [1], f32, kind="ExternalInput").ap(),
        nc.dram_tensor("out_lp", [N, N, H], f32, kind="ExternalOutput").ap(),
        nc.dram_tensor("out_gp", [N, N, H], f32, kind="ExternalOutput").ap(),
    )
    with tile.TileContext(nc) as tc:
        _body(tc, io, reps=reps)
    nc.compile()
    return nc


def _get_nc():
    if "nc" not in _cache:
        _cache["nc"] = _build()
    return _cache["nc"]


def _run(inputs, trace=False):
    from concourse.bass_utils import run_bass_kernel_spmd

    nc = _get_nc()
    f = lambda x: np.ascontiguousarray(np.asarray(x), dtype=np.float32)
    shared = {
        "w_apair": f(inputs["W_apair"]),
        "b_apair": f(inputs["b_apair"]),
        "w_binary": f(inputs["W_binary"]),
        "b_binary": f(inputs["b_binary"]),
        "w_att": f(inputs["W_att"]),
        "b_att": f(inputs["b_att"]),
    }
    local = f(inputs["local_feats"])
    binary = f(inputs["binary_feats"])
    in_maps = [
        {"local": local[c], "binary": binary[c], **shared} for c in range(NCORES)
    ]
    res = run_bass_kernel_spmd(
        nc, in_maps, core_ids=list(range(NCORES)), trace=trace
    )
    lp = np.stack([r["out_lp"] for r in res.results])
    gp = np.stack([r["out_gp"] for r in res.results])
    return (lp, gp), res


def kernel(**inputs):
    out, _ = _run(inputs, trace=False)
    return out


# revision 18
# speedup vs baseline: 2.2279x; 2.2279x over previous
"""Trainium2 Bass kernel for nn_Attention_54580444397738 (gnn_message_passing).

Math per batch b (B=8, N=128, H=256, C=16):
  proj         = local @ W_apair                                     [N, H]
  pre[i,j,:]   = proj[i,:] + proj[j,:] + binary[i,j,:] @ W_binary
                 + b_apair + b_binary                                [N, N, H]
  score[i,j]   = sigmoid(relu(pre[i,j,:]) . W_att + b_att)           [N, N]
  glob         = score @ local                                       [N, H]
  local_pair [i,j,:] = local[i,:] + local[j,:]                       (output 1)
  global_pair[i,j,:] = glob[i,:]  + glob[j,:]                        (output 2)

Key algebraic simplification: einsum("bijh,hk->bijk", local_pair, W_apair)
= proj[i,:] + proj[j,:], so the N^2xHxH matmul collapses to an NxHxH one.

Sharding: data-parallel over batch B across the 8 cores (1 batch per core).
The outputs (2 x 16 MB fp32 per core) dominate -> memory-bound.

Implementation notes:
  - All attention matmuls use float32r (TF32-like, 1 cycle/row vs 4 for
    fp32); the ~1e-4 rounding noise is attenuated through sigmoid to ~2e-5
    on global_pair.
  - Output tiles [j=128, h=256] = X[j,:] + X[i,:] are built exactly:
    PE broadcasts rows via ones-matmuls in f32r using a compensated pair
    (rowR = f32r(x), rowE = f32r(x - rowR); error ~1e-8), or GPSIMD
    partition_broadcast (bit-exact); DVE/ACT assemble into SBUF staging,
    8 tiles per 1 MB DMA store.
  - Variant mix per tile ('A' PE+ACT, 'B' PE+DVE-copy, 'C' PE-rows+DVE-add,
    'G' POOL-bcast+DVE-add) balances the four engines.
"""

import numpy as np

B, N, H, BIN = 8, 128, 256, 16
NCORES = 8
CPAD = 32        # c dim padded 16 -> 32 so transposed blocks land 32-aligned
IG = 4           # i's per binary-transpose group (4 * 32 = 128)
STAGE_I = 8      # output tiles per staged 1MB DMA store
PROLOG = 16      # local_pair tiles emitted before attention work starts

# per-16-tile variant patterns (A: PE-psum+ACT-copy, B: PE-psum+DVE-copy,
# C: PE-row-psum+DVE-add, G: POOL-bcast+DVE-add)
VAR_LP = "AACAGACACAGCAACG"
VAR_GP = "AACAGACACAGCAACG"

_cache = {}


def _body(tc, io, reps=1):
    import concourse.bass as bass
    import concourse.mybir as mybir
    from concourse.masks import make_identity
    from contextlib import ExitStack, nullcontext

    nc = tc.nc
    ts = bass.ts
    f32 = mybir.dt.float32
    f32r = mybir.dt.float32r
    Relu = mybir.ActivationFunctionType.Relu
    Sigmoid = mybir.ActivationFunctionType.Sigmoid

    local_d, binary_d, wap_d, bap_d, wbin_d, bbin_d, watt_d, batt_d, lp_d, gp_d = io

    ctx = ExitStack()
    with ctx:
        persist = ctx.enter_context(tc.tile_pool(name="persist", bufs=1))
        binTp = ctx.enter_context(tc.tile_pool(name="binTp", bufs=6))
        att2p = ctx.enter_context(tc.tile_pool(name="att2p", bufs=3))
        stagep = ctx.enter_context(tc.tile_pool(name="stagep", bufs=2))
        bcastp = ctx.enter_context(tc.tile_pool(name="bcastp", bufs=3))
        prep = ctx.enter_context(tc.tile_pool(name="prep", bufs=2, space="PSUM"))
        outpp = ctx.enter_context(tc.tile_pool(name="outpp", bufs=4, space="PSUM"))
        dramp = ctx.enter_context(tc.tile_pool(name="dramp", bufs=1, space="DRAM"))

        # timing builds wrap the whole body in a device-side loop
        loop = tc.For_i(0, reps, 1) if reps > 1 else nullcontext()
        ctx.enter_context(loop)

        # ---------------- persistent setup ----------------
        identity = persist.tile([128, 128], f32, tag="identity")
        make_identity(nc, identity)
        identR = persist.tile([128, 128], f32r, tag="identR")
        nc.vector.tensor_copy(out=identR, in_=identity)
        onesF = persist.tile([128, 128], f32, tag="onesF")
        nc.gpsimd.memset(onesF, 1.0)
        onesT = persist.tile([128, 128], f32r, tag="onesT")
        nc.vector.tensor_copy(out=onesT, in_=onesF)

        localSb = persist.tile([N, H], f32, tag="localSb")
        nc.sync.dma_start(out=localSb, in_=local_d)

        # f32r weights (cast during SWDGE load)
        wap0 = persist.tile([128, H], f32r, tag="wap0")
        nc.gpsimd.dma_start(out=wap0, in_=wap_d[0:128])
        wap1 = persist.tile([128, H], f32r, tag="wap1")
        nc.gpsimd.dma_start(out=wap1, in_=wap_d[128:256])

        biasA = persist.tile([1, H], f32, tag="biasA")
        nc.sync.dma_start(out=biasA, in_=bap_d.unsqueeze(0))
        biasB = persist.tile([1, H], f32, tag="biasB")
        nc.sync.dma_start(out=biasB, in_=bbin_d.unsqueeze(0))
        biasRow = persist.tile([1, H], f32r, tag="biasRow")
        nc.vector.tensor_add(out=biasRow, in0=biasA, in1=biasB)

        wbinR = persist.tile([16, H], f32r, tag="wbinR")
        nc.gpsimd.dma_start(out=wbinR, in_=wbin_d)

        wattRow = persist.tile([1, H], f32, tag="wattRow")
        nc.sync.dma_start(out=wattRow, in_=watt_d.rearrange("k o -> o k"))
        battRow = persist.tile([1, 1], f32, tag="battRow")
        nc.sync.dma_start(out=battRow, in_=batt_d.unsqueeze(0))

        # Wx4: W_binary + bias row replicated at partitions {0,32,64,96}
        wx4 = persist.tile([128, H], f32r, tag="wx4")
        for m in range(4):
            nc.sync.dma_start(out=wx4[32 * m : 32 * m + 16, :], in_=wbinR)
            nc.sync.dma_start(out=wx4[32 * m + 16 : 32 * m + 17, :], in_=biasRow)

        # broadcast W_att across partitions; b_att as a [128,1] column
        wattB = persist.tile([128, H], f32, tag="wattB")
        battCol = persist.tile([128, 1], f32, tag="battCol")
        nc.gpsimd.partition_broadcast(wattB, wattRow)
        nc.gpsimd.partition_broadcast(battCol, battRow)

        # localT = local^T (f32r), then projW = local @ W_apair (f32r)
        localT = persist.tile([128, H], f32r, tag="localT")
        for hb in range(2):
            tp = outpp.tile([128, H], f32, tag="outp")
            nc.tensor.transpose(tp[:, 0:128], localSb[:, ts(hb, 128)], identity)
            nc.scalar.copy(out=localT[:, ts(hb, 128)], in_=tp[:, 0:128])
        pp = outpp.tile([128, H], f32, tag="outp")
        nc.tensor.matmul(pp, lhsT=localT[:, 0:128], rhs=wap0, start=True, stop=False)
        nc.tensor.matmul(pp, lhsT=localT[:, 128:256], rhs=wap1, start=False, stop=True)
        projW = persist.tile([128, H], f32r, tag="projW")
        nc.scalar.copy(out=projW, in_=pp)

        projDram = dramp.tile([N, H], f32r, tag="projDram")
        nc.sync.dma_start(out=projDram, in_=projW)
        projFlat = persist.tile([97, 32 * H], f32r, tag="projFlat")
        pf4 = projDram.rearrange("(a x) h -> a (x h)", a=4)
        for q in range(4):
            nc.sync.dma_start(out=projFlat[32 * q : 32 * q + 1, :], in_=pf4[q : q + 1])

        # compensated f32r split of X for exact PE broadcasts:
        #   XR = f32r(X), XE = f32r(X - XR);  XR + XE == X to ~1e-8
        # flatX holds exact f32 rows (for the G-variant POOL broadcast).
        xR = persist.tile([N, H], f32r, tag="xR")
        xE = persist.tile([N, H], f32r, tag="xE")
        flatR = persist.tile([97, 32 * H], f32r, tag="flatR")
        flatE = persist.tile([97, 32 * H], f32r, tag="flatE")
        xRDram = dramp.tile([N, H], f32r, tag="xRDram")
        xEDram = dramp.tile([N, H], f32r, tag="xEDram")

        def split_x(xSb):
            """fill xR/xE from xSb and bounce rows into flatR/flatE"""
            nc.vector.tensor_copy(out=xR, in_=xSb)
            nc.vector.tensor_sub(out=xE, in0=xSb, in1=xR.bitcast(f32))
            nc.sync.dma_start(out=xRDram, in_=xR)
            nc.sync.dma_start(out=xEDram, in_=xE)
            r4 = xRDram.rearrange("(a x) h -> a (x h)", a=4)
            e4 = xEDram.rearrange("(a x) h -> a (x h)", a=4)
            for q in range(4):
                nc.sync.dma_start(out=flatR[32 * q : 32 * q + 1, :], in_=r4[q : q + 1])
                nc.sync.dma_start(out=flatE[32 * q : 32 * q + 1, :], in_=e4[q : q + 1])

        split_x(localSb)

        # binp[j, (i, c32)]: c 0..15 = binary[., i, j, .], c16 = 1.0 (bias lane)
        binp = persist.tile([128, N * CPAD], f32, tag="binp")
        nc.gpsimd.memset(binp, 0.0)
        binp3 = binp.rearrange("p (i c) -> p i c", c=CPAD)
        nc.gpsimd.memset(binp3[:, :, 16:17], 1.0)
        for q in range(4):
            nc.sync.dma_start(
                out=binp3[:, ts(q, 32), 0:BIN],
                in_=binary_d[ts(q, 32)].rearrange("i j c -> j i c"),
            )

        logits = persist.tile([128, N], f32, tag="logits")
        ttrS = persist.tile([128, H], f32, tag="ttrS")
        binT = {}

        # ---------------- helpers ----------------
        def row_rhs(flat, i, width):
            q, r = divmod(i, 32)
            return flat[32 * q : 32 * q + 1, r * H : r * H + width]

        def row_lhsT(i):
            q = i // 32
            return onesT[32 * q : 32 * q + 1, :]

        def row_tp(i):
            return (32 * (i // 32), 0)

        def out_tile(i, xSb, x_dram, stage, variant):
            dst = stage[:, ts(i % STAGE_I, H)]
            if variant in ("A", "B"):
                po = outpp.tile([128, H], f32, tag="outp")
                nc.tensor.matmul(po, lhsT=identR, rhs=xR, start=True, stop=False)
                nc.tensor.matmul(po, lhsT=identR, rhs=xE, start=False, stop=False)
                nc.tensor.matmul(po, lhsT=row_lhsT(i), rhs=row_rhs(flatR, i, H),
                                 start=False, stop=False, tile_position=row_tp(i))
                nc.tensor.matmul(po, lhsT=row_lhsT(i), rhs=row_rhs(flatE, i, H),
                                 start=False, stop=True, tile_position=row_tp(i))
                if variant == "A":
                    nc.scalar.copy(out=dst, in_=po)
                else:
                    nc.vector.tensor_copy(out=dst, in_=po)
            elif variant == "C":
                po = outpp.tile([128, H], f32, tag="outp")
                nc.tensor.matmul(po, lhsT=row_lhsT(i), rhs=row_rhs(flatR, i, H),
                                 start=True, stop=False, tile_position=row_tp(i))
                nc.tensor.matmul(po, lhsT=row_lhsT(i), rhs=row_rhs(flatE, i, H),
                                 start=False, stop=True, tile_position=row_tp(i))
                nc.vector.tensor_add(out=dst, in0=xSb, in1=po)
            else:  # 'G': bit-exact POOL broadcast + DVE add
                # HW partition_broadcast only honors partition-0 sources, so
                # bounce the row from DRAM into a partition-0 tile first.
                br = bcastp.tile([1, H], f32, tag="bcrow")
                nc.sync.dma_start(out=br, in_=x_dram[i : i + 1, :])
                bt = bcastp.tile([128, H], f32, tag="bc")
                nc.gpsimd.partition_broadcast(bt, br)
                nc.vector.tensor_add(out=dst, in0=xSb, in1=bt)

        def flush_stage(i0, stage, dram_out):
            nc.sync.dma_start(
                out=dram_out[i0 : i0 + STAGE_I].rearrange("i j h -> j i h"),
                in_=stage.rearrange("p (i h) -> p i h", h=H),
            )

        def attn_step(i):
            g, il = divmod(i, IG)
            if il == 0:  # transpose this binary group: [j,(i4,c32)] -> [(i4,c32),j]
                tp = outpp.tile([128, H], f32, tag="outp")
                nc.tensor.transpose(tp[:, 0:128], binp[:, ts(g, 128)], identity)
                bt = binTp.tile([128, 128], f32r, tag="binT")
                nc.scalar.copy(out=bt, in_=tp[:, 0:128])
                binT[g] = bt
            if i % 2 == 1:
                return
            pre = prep.tile([128, 2 * H], f32, tag="pre")
            nc.tensor.matmul(pre, lhsT=row_lhsT(i), rhs=row_rhs(projFlat, i, 2 * H),
                             start=True, stop=False, tile_position=row_tp(i))
            for m in range(2):
                ii = i + m
                gg, iil = divmod(ii, IG)
                nc.tensor.matmul(pre[:, ts(m, H)], lhsT=identR, rhs=projW,
                                 start=False, stop=False)
                nc.tensor.matmul(
                    pre[:, ts(m, H)],
                    lhsT=binT[gg][32 * iil : 32 * iil + 17, :],
                    rhs=wx4[32 * iil : 32 * iil + 17, :],
                    start=False, stop=(m == 1), tile_position=(32 * iil, 0),
                )
            a2 = att2p.tile([128, 2 * H], f32, tag="att2")
            nc.scalar.activation(out=a2, in_=pre, func=Relu)
            for m in range(2):
                nc.vector.affine_mul_reduce(
                    out=ttrS, accum_out=logits[:, i + m : i + m + 1],
                    in0=a2[:, ts(m, H)], in1=wattB, scale=1.0, bias=0.0,
                )

        # ---------------- phase 1: local_pair + attention ----------------
        stage = None
        for i in range(N):
            if i % STAGE_I == 0:
                stage = stagep.tile([128, STAGE_I * H], f32, tag="stage")
            out_tile(i, localSb, local_d, stage, VAR_LP[i % len(VAR_LP)])
            if i % STAGE_I == STAGE_I - 1:
                flush_stage(i - STAGE_I + 1, stage, lp_d)
            if i >= PROLOG:
                attn_step(i - PROLOG)
        for i in range(N - PROLOG, N):
            attn_step(i)

        # ---------------- scores -> glob ----------------
        scoreT = persist.tile([128, N], f32, tag="scoreT")
        nc.scalar.activation(out=scoreT, in_=logits, func=Sigmoid, bias=battCol)
        pg = outpp.tile([128, H], f32, tag="outp")
        nc.tensor.matmul(pg, lhsT=scoreT, rhs=localSb, start=True, stop=True)
        globSb = persist.tile([128, H], f32, tag="globSb")
        nc.vector.tensor_copy(out=globSb, in_=pg)
        globDram = dramp.tile([N, H], f32, tag="globDram")
        nc.sync.dma_start(out=globDram, in_=globSb)
        split_x(globSb)

        # ---------------- phase 2: global_pair ----------------
        for i in range(N):
            if i % STAGE_I == 0:
                stage = stagep.tile([128, STAGE_I * H], f32, tag="stage")
            out_tile(i, globSb, globDram, stage, VAR_GP[i % len(VAR_GP)])
            if i % STAGE_I == STAGE_I - 1:
                flush_stage(i - STAGE_I + 1, stage, gp_d)


def _build(reps=1):
    import concourse.bass as bass  # noqa: F401
    from concourse import bacc
    import concourse.mybir as mybir
    import concourse.tile as tile

    f32 = mybir.dt.float32
    nc = bacc.Bacc(
        "TRN2",
        target_bir_lowering=False,
        debug=False,
        enable_asserts=False,
        num_devices=NCORES,
    )
    io = (
        nc.dram_tensor("local", [N, H], f32, kind="ExternalInput").ap(),
        nc.dram_tensor("binary", [N, N, BIN], f32, kind="ExternalInput").ap(),
        nc.dram_tensor("w_apair", [H, H], f32, kind="ExternalInput").ap(),
        nc.dram_tensor("b_apair", [H], f32, kind="ExternalInput").ap(),
        nc.dram_tensor("w_binary", [BIN, H], f32, kind="ExternalInput").ap(),
        nc.dram_tensor("b_binary", [H], f32, kind="ExternalInput").ap(),
        nc.dram_tensor("w_att", [H, 1], f32, kind="ExternalInput").ap(),
        nc.dram_tensor("b_att", [1], f32, kind="ExternalInput").ap(),
        nc.dram_tensor("out_lp", [N, N, H], f32, kind="ExternalOutput").ap(),
        nc.dram_tensor("out_gp", [N, N, H], f32, kind="ExternalOutput").ap(),
    )
    with tile.TileContext(nc) as tc:
        _body(tc, io, reps=reps)
    nc.compile()
    return nc


def _get_nc():
    if "nc" not in _cache:
        _cache["nc"] = _build()
    return _cache["nc"]


def _run(inputs, trace=False):
    from concourse.bass_utils import run_bass_kernel_spmd

    nc = _get_nc()
    f = lambda x: np.ascontiguousarray(np.asarray(x), dtype=np.float32)
    shared = {
        "w_apair": f(inputs["W_apair"]),
        "b_apair": f(inputs["b_apair"]),
        "w_binary": f(inputs["W_binary"]),
        "b_binary": f(inputs["b_binary"]),
        "w_att": f(inputs["W_att"]),
        "b_att": f(inputs["b_att"]),
    }
    local = f(inputs["local_feats"])
    binary = f(inputs["binary_feats"])
    in_maps = [
        {"local": local[c], "binary": binary[c], **shared} for c in range(NCORES)
    ]
    res = run_bass_kernel_spmd(
        nc, in_maps, core_ids=list(range(NCORES)), trace=trace
    )
    lp = np.stack([r["out_lp"] for r in res.results])
    gp = np.stack([r["out_gp"] for r in res.results])
    return (lp, gp), res


def kernel(**inputs):
    out, _ = _run(inputs, trace=False)
    return out


# revision 26
# speedup vs baseline: 2.3588x; 1.0587x over previous
"""Trainium2 Bass kernel for nn_Attention_54580444397738 (gnn_message_passing).

Math per batch b (B=8, N=128, H=256, C=16):
  proj         = local @ W_apair                                     [N, H]
  pre[i,j,:]   = proj[i,:] + proj[j,:] + binary[i,j,:] @ W_binary
                 + b_apair + b_binary                                [N, N, H]
  score[i,j]   = sigmoid(relu(pre[i,j,:]) . W_att + b_att)           [N, N]
  glob         = score @ local                                       [N, H]
  local_pair [i,j,:] = local[i,:] + local[j,:]                       (output 1)
  global_pair[i,j,:] = glob[i,:]  + glob[j,:]                        (output 2)

Key algebraic simplification: einsum("bijh,hk->bijk", local_pair, W_apair)
= proj[i,:] + proj[j,:], so the N^2xHxH matmul collapses to an NxHxH one.

Sharding: data-parallel over batch B across the 8 cores (1 batch per core).
The outputs (2 x 16 MB fp32 per core) dominate -> memory-bound.

Implementation notes:
  - All attention matmuls use float32r (TF32-like, 1 cycle/row vs 4 for
    fp32); the ~1e-4 rounding noise is attenuated through sigmoid to ~2e-5
    on global_pair.
  - Output tiles [j=128, h=256] = X[j,:] + X[i,:] are built exactly:
    PE broadcasts rows via ones-matmuls in f32r using a compensated pair
    (rowR = f32r(x), rowE = f32r(x - rowR); error ~1e-8), or GPSIMD
    partition_broadcast (bit-exact); DVE/ACT assemble into SBUF staging,
    8 tiles per 1 MB DMA store.
  - Variant mix per tile ('A' PE+ACT, 'B' PE+DVE-copy, 'C' PE-rows+DVE-add,
    'G' POOL-bcast+DVE-add) balances the four engines.
"""

import numpy as np

B, N, H, BIN = 8, 128, 256, 16
NCORES = 8
CPAD = 32        # c dim padded 16 -> 32 so transposed blocks land 32-aligned
IG = 4           # i's per binary-transpose group (4 * 32 = 128)
STAGE_I = 8      # output tiles per staged 1MB DMA store
PROLOG = 16      # local_pair tiles emitted before attention work starts

# variant per tile (A: PE-psum+ACT-copy, C: PE-row-psum+DVE-add,
# G: POOL-bcast+DVE-add). G positions are arithmetic (i%16 = 2+3k) so all
# G rows of a phase load with a single strided DMA.
VAR_PAT = "ACGACGCAGACGCACG"


def variant_of(i):
    return VAR_PAT[i % 16]

_cache = {}


def _body(tc, io, reps=1):
    import concourse.bass as bass
    import concourse.mybir as mybir
    from concourse.masks import make_identity
    from contextlib import ExitStack, nullcontext

    nc = tc.nc
    ts = bass.ts
    f32 = mybir.dt.float32
    f32r = mybir.dt.float32r
    Relu = mybir.ActivationFunctionType.Relu
    Sigmoid = mybir.ActivationFunctionType.Sigmoid

    local_d, binary_d, wap_d, bap_d, wbin_d, bbin_d, watt_d, batt_d, lp_d, gp_d = io

    ctx = ExitStack()
    with ctx:
        persist = ctx.enter_context(tc.tile_pool(name="persist", bufs=1))
        binTp = ctx.enter_context(tc.tile_pool(name="binTp", bufs=6))
        att2p = ctx.enter_context(tc.tile_pool(name="att2p", bufs=3))
        stagep = ctx.enter_context(tc.tile_pool(name="stagep", bufs=2))
        bcastp = ctx.enter_context(tc.tile_pool(name="bcastp", bufs=3))
        prep = ctx.enter_context(tc.tile_pool(name="prep", bufs=2, space="PSUM"))
        outpp = ctx.enter_context(tc.tile_pool(name="outpp", bufs=4, space="PSUM"))
        dramp = ctx.enter_context(tc.tile_pool(name="dramp", bufs=1, space="DRAM"))

        # timing builds wrap the whole body in a device-side loop
        loop = tc.For_i(0, reps, 1) if reps > 1 else nullcontext()
        ctx.enter_context(loop)

        # ---------------- persistent setup ----------------
        identity = persist.tile([128, 128], f32, tag="identity")
        make_identity(nc, identity)
        identR = persist.tile([128, 128], f32r, tag="identR")
        nc.vector.tensor_copy(out=identR, in_=identity)
        onesF = persist.tile([128, 128], f32, tag="onesF")
        nc.gpsimd.memset(onesF, 1.0)
        onesT = persist.tile([128, 128], f32r, tag="onesT")
        nc.vector.tensor_copy(out=onesT, in_=onesF)

        localSb = persist.tile([N, H], f32, tag="localSb")
        nc.sync.dma_start(out=localSb, in_=local_d)

        # f32r weights (cast during SWDGE load)
        wap0 = persist.tile([128, H], f32r, tag="wap0")
        nc.gpsimd.dma_start(out=wap0, in_=wap_d[0:128])
        wap1 = persist.tile([128, H], f32r, tag="wap1")
        nc.gpsimd.dma_start(out=wap1, in_=wap_d[128:256])

        biasA = persist.tile([1, H], f32, tag="biasA")
        nc.sync.dma_start(out=biasA, in_=bap_d.unsqueeze(0))
        biasB = persist.tile([1, H], f32, tag="biasB")
        nc.sync.dma_start(out=biasB, in_=bbin_d.unsqueeze(0))
        biasRow = persist.tile([1, H], f32r, tag="biasRow")
        nc.vector.tensor_add(out=biasRow, in0=biasA, in1=biasB)

        wbinR = persist.tile([16, H], f32r, tag="wbinR")
        nc.gpsimd.dma_start(out=wbinR, in_=wbin_d)

        wattRow = persist.tile([1, H], f32, tag="wattRow")
        nc.sync.dma_start(out=wattRow, in_=watt_d.rearrange("k o -> o k"))
        battRow = persist.tile([1, 1], f32, tag="battRow")
        nc.sync.dma_start(out=battRow, in_=batt_d.unsqueeze(0))

        # Wx4: W_binary + bias row replicated at partitions {0,32,64,96}
        wx4 = persist.tile([128, H], f32r, tag="wx4")
        for m in range(4):
            nc.sync.dma_start(out=wx4[32 * m : 32 * m + 16, :], in_=wbinR)
            nc.sync.dma_start(out=wx4[32 * m + 16 : 32 * m + 17, :], in_=biasRow)

        # broadcast W_att across partitions; b_att as a [128,1] column
        wattB = persist.tile([128, H], f32, tag="wattB")
        battCol = persist.tile([128, 1], f32, tag="battCol")
        nc.gpsimd.partition_broadcast(wattB, wattRow)
        nc.gpsimd.partition_broadcast(battCol, battRow)

        # localT = local^T (f32r), then projW = local @ W_apair (f32r)
        localT = persist.tile([128, H], f32r, tag="localT")
        for hb in range(2):
            tp = outpp.tile([128, H], f32, tag="outp")
            nc.tensor.transpose(tp[:, 0:128], localSb[:, ts(hb, 128)], identity)
            nc.scalar.copy(out=localT[:, ts(hb, 128)], in_=tp[:, 0:128])
        pp = outpp.tile([128, H], f32, tag="outp")
        nc.tensor.matmul(pp, lhsT=localT[:, 0:128], rhs=wap0, start=True, stop=False)
        nc.tensor.matmul(pp, lhsT=localT[:, 128:256], rhs=wap1, start=False, stop=True)
        projW = persist.tile([128, H], f32r, tag="projW")
        nc.scalar.copy(out=projW, in_=pp)

        projDram = dramp.tile([N, H], f32r, tag="projDram")
        nc.sync.dma_start(out=projDram, in_=projW)
        projFlat = persist.tile([97, 32 * H], f32r, tag="projFlat")
        pf4 = projDram.rearrange("(a x) h -> a (x h)", a=4)
        for q in range(4):
            nc.sync.dma_start(out=projFlat[32 * q : 32 * q + 1, :], in_=pf4[q : q + 1])

        # compensated f32r split of X for exact PE broadcasts:
        #   XR = f32r(X), XE = f32r(X - XR);  XR + XE == X to ~1e-8
        # flatX holds exact f32 rows (for the G-variant POOL broadcast).
        xR = persist.tile([N, H], f32r, tag="xR")
        xE = persist.tile([N, H], f32r, tag="xE")
        flatR = persist.tile([97, 32 * H], f32r, tag="flatR")
        flatE = persist.tile([97, 32 * H], f32r, tag="flatE")
        xRDram = dramp.tile([N, H], f32r, tag="xRDram")
        xEDram = dramp.tile([N, H], f32r, tag="xEDram")

        def split_x(xSb):
            """fill xR/xE from xSb and bounce rows into flatR/flatE"""
            nc.vector.tensor_copy(out=xR, in_=xSb)
            nc.vector.tensor_sub(out=xE, in0=xSb, in1=xR.bitcast(f32))
            nc.sync.dma_start(out=xRDram, in_=xR)
            nc.sync.dma_start(out=xEDram, in_=xE)
            r4 = xRDram.rearrange("(a x) h -> a (x h)", a=4)
            e4 = xEDram.rearrange("(a x) h -> a (x h)", a=4)
            for q in range(4):
                nc.sync.dma_start(out=flatR[32 * q : 32 * q + 1, :], in_=r4[q : q + 1])
                nc.sync.dma_start(out=flatE[32 * q : 32 * q + 1, :], in_=e4[q : q + 1])

        split_x(localSb)

        # binp[j, (i, c32)]: c 0..15 = binary[., i, j, .], c16 = 1.0 (bias lane)
        binp = persist.tile([128, N * CPAD], f32, tag="binp")
        nc.gpsimd.memset(binp, 0.0)
        binp3 = binp.rearrange("p (i c) -> p i c", c=CPAD)
        nc.gpsimd.memset(binp3[:, :, 16:17], 1.0)
        for q in range(4):
            nc.sync.dma_start(
                out=binp3[:, ts(q, 32), 0:BIN],
                in_=binary_d[ts(q, 32)].rearrange("i j c -> j i c"),
            )

        logits = persist.tile([128, N], f32, tag="logits")
        ttrS = persist.tile([128, H], f32, tag="ttrS")
        binT = {}

        # ---------------- helpers ----------------
        def row_rhs(flat, i, width):
            q, r = divmod(i, 32)
            return flat[32 * q : 32 * q + 1, r * H : r * H + width]

        def row_lhsT(i):
            q = i // 32
            return onesT[32 * q : 32 * q + 1, :]

        def row_tp(i):
            return (32 * (i // 32), 0)

        def out_tile(i, xSb, x_dram, stage, variant):
            dst = stage[:, ts(i % STAGE_I, H)]
            if variant in ("A", "B"):
                po = outpp.tile([128, H], f32, tag="outp")
                nc.tensor.matmul(po, lhsT=identR, rhs=xR, start=True, stop=False)
                nc.tensor.matmul(po, lhsT=identR, rhs=xE, start=False, stop=False)
                nc.tensor.matmul(po, lhsT=row_lhsT(i), rhs=row_rhs(flatR, i, H),
                                 start=False, stop=False, tile_position=row_tp(i))
                nc.tensor.matmul(po, lhsT=row_lhsT(i), rhs=row_rhs(flatE, i, H),
                                 start=False, stop=True, tile_position=row_tp(i))
                if variant == "A":
                    nc.scalar.copy(out=dst, in_=po)
                else:
                    nc.vector.tensor_copy(out=dst, in_=po)
            elif variant == "C":
                po = outpp.tile([128, H], f32, tag="outp")
                nc.tensor.matmul(po, lhsT=row_lhsT(i), rhs=row_rhs(flatR, i, H),
                                 start=True, stop=False, tile_position=row_tp(i))
                nc.tensor.matmul(po, lhsT=row_lhsT(i), rhs=row_rhs(flatE, i, H),
                                 start=False, stop=True, tile_position=row_tp(i))
                nc.vector.tensor_add(out=dst, in0=xSb, in1=po)
            else:  # 'G': bit-exact POOL broadcast + DVE add
                br = bcastp.tile([1, H], f32, tag="bcrow")
                nc.sync.dma_start(out=br, in_=x_dram[i : i + 1, :])
                bt = bcastp.tile([128, H], f32, tag="bc")
                nc.gpsimd.partition_broadcast(bt, br)
                nc.vector.tensor_add(out=dst, in0=xSb, in1=bt)

        def flush_stage(i0, stage, dram_out):
            nc.sync.dma_start(
                out=dram_out[i0 : i0 + STAGE_I].rearrange("i j h -> j i h"),
                in_=stage.rearrange("p (i h) -> p i h", h=H),
            )

        def attn_step(i):
            g, il = divmod(i, IG)
            if il == 0:  # transpose this binary group: [j,(i4,c32)] -> [(i4,c32),j]
                tp = outpp.tile([128, H], f32, tag="outp")
                nc.tensor.transpose(tp[:, 0:128], binp[:, ts(g, 128)], identity)
                bt = binTp.tile([128, 128], f32r, tag="binT")
                nc.scalar.copy(out=bt, in_=tp[:, 0:128])
                binT[g] = bt
            if i % 2 == 1:
                return
            pre = prep.tile([128, 2 * H], f32, tag="pre")
            nc.tensor.matmul(pre, lhsT=row_lhsT(i), rhs=row_rhs(projFlat, i, 2 * H),
                             start=True, stop=False, tile_position=row_tp(i))
            for m in range(2):
                ii = i + m
                gg, iil = divmod(ii, IG)
                nc.tensor.matmul(pre[:, ts(m, H)], lhsT=identR, rhs=projW,
                                 start=False, stop=False)
                nc.tensor.matmul(
                    pre[:, ts(m, H)],
                    lhsT=binT[gg][32 * iil : 32 * iil + 17, :],
                    rhs=wx4[32 * iil : 32 * iil + 17, :],
                    start=False, stop=(m == 1), tile_position=(32 * iil, 0),
                )
            a2 = att2p.tile([128, 2 * H], f32, tag="att2")
            nc.scalar.activation(out=a2, in_=pre, func=Relu)
            for m in range(2):
                nc.vector.affine_mul_reduce(
                    out=ttrS, accum_out=logits[:, i + m : i + m + 1],
                    in0=a2[:, ts(m, H)], in1=wattB, scale=1.0, bias=0.0,
                )

        # ---------------- phase 1: local_pair + attention ----------------
        stage = None
        for i in range(N):
            if i % STAGE_I == 0:
                stage = stagep.tile([128, STAGE_I * H], f32, tag="stage")
            out_tile(i, localSb, local_d, stage, variant_of(i))
            if i % STAGE_I == STAGE_I - 1:
                flush_stage(i - STAGE_I + 1, stage, lp_d)
            if i >= PROLOG:
                attn_step(i - PROLOG)
        for i in range(N - PROLOG, N):
            attn_step(i)

        # ---------------- scores -> glob ----------------
        scoreT = persist.tile([128, N], f32, tag="scoreT")
        nc.scalar.activation(out=scoreT, in_=logits, func=Sigmoid, bias=battCol)
        pg = outpp.tile([128, H], f32, tag="outp")
        nc.tensor.matmul(pg, lhsT=scoreT, rhs=localSb, start=True, stop=True)
        globSb = persist.tile([128, H], f32, tag="globSb")
        nc.vector.tensor_copy(out=globSb, in_=pg)
        globDram = dramp.tile([N, H], f32, tag="globDram")
        nc.sync.dma_start(out=globDram, in_=globSb)
        split_x(globSb)

        # ---------------- phase 2: global_pair ----------------
        for i in range(N):
            if i % STAGE_I == 0:
                stage = stagep.tile([128, STAGE_I * H], f32, tag="stage")
            out_tile(i, globSb, globDram, stage, variant_of(i))
            if i % STAGE_I == STAGE_I - 1:
                flush_stage(i - STAGE_I + 1, stage, gp_d)


def _build(reps=1):
    import concourse.bass as bass  # noqa: F401
    from concourse import bacc
    import concourse.mybir as mybir
    import concourse.tile as tile

    f32 = mybir.dt.float32
    nc = bacc.Bacc(
        "TRN2",
        target_bir_lowering=False,
        debug=False,
        enable_asserts=False,
        num_devices=NCORES,
    )
    io = (
        nc.dram_tensor("local", [N, H], f32, kind="ExternalInput").ap(),
        nc.dram_tensor("binary", [N, N, BIN], f32, kind="ExternalInput").ap(),
        nc.dram_tensor("w_apair", [H, H], f32, kind="ExternalInput").ap(),
        nc.dram_tensor("b_apair", [H], f32, kind="ExternalInput").ap(),
        nc.dram_tensor("w_binary", [BIN, H], f32, kind="ExternalInput").ap(),
        nc.dram_tensor("b_binary", [H], f32, kind="ExternalInput").ap(),
        nc.dram_tensor("w_att", [H, 1], f32, kind="ExternalInput").ap(),
        nc.dram_tensor("b_att", [1], f32, kind="ExternalInput").ap(),
        nc.dram_tensor("out_lp", [N, N, H], f32, kind="ExternalOutput").ap(),
        nc.dram_tensor("out_gp", [N, N, H], f32, kind="ExternalOutput").ap(),
    )
    with tile.TileContext(nc) as tc:
        _body(tc, io, reps=reps)
    nc.compile()
    return nc


def _get_nc():
    if "nc" not in _cache:
        _cache["nc"] = _build()
    return _cache["nc"]


def _run(inputs, trace=False):
    from concourse.bass_utils import run_bass_kernel_spmd

    nc = _get_nc()
    f = lambda x: np.ascontiguousarray(np.asarray(x), dtype=np.float32)
    shared = {
        "w_apair": f(inputs["W_apair"]),
        "b_apair": f(inputs["b_apair"]),
        "w_binary": f(inputs["W_binary"]),
        "b_binary": f(inputs["b_binary"]),
        "w_att": f(inputs["W_att"]),
        "b_att": f(inputs["b_att"]),
    }
    local = f(inputs["local_feats"])
    binary = f(inputs["binary_feats"])
    in_maps = [
        {"local": local[c], "binary": binary[c], **shared} for c in range(NCORES)
    ]
    res = run_bass_kernel_spmd(
        nc, in_maps, core_ids=list(range(NCORES)), trace=trace
    )
    lp = np.stack([r["out_lp"] for r in res.results])
    gp = np.stack([r["out_gp"] for r in res.results])
    return (lp, gp), res


def kernel(**inputs):
    out, _ = _run(inputs, trace=False)
    return out


# revision 29
# speedup vs baseline: 2.3912x; 1.0138x over previous
"""Trainium2 Bass kernel for nn_Attention_54580444397738 (gnn_message_passing).

Math per batch b (B=8, N=128, H=256, C=16):
  proj         = local @ W_apair                                     [N, H]
  pre[i,j,:]   = proj[i,:] + proj[j,:] + binary[i,j,:] @ W_binary
                 + b_apair + b_binary                                [N, N, H]
  score[i,j]   = sigmoid(relu(pre[i,j,:]) . W_att + b_att)           [N, N]
  glob         = score @ local                                       [N, H]
  local_pair [i,j,:] = local[i,:] + local[j,:]                       (output 1)
  global_pair[i,j,:] = glob[i,:]  + glob[j,:]                        (output 2)

Key algebraic simplification: einsum("bijh,hk->bijk", local_pair, W_apair)
= proj[i,:] + proj[j,:], so the N^2xHxH matmul collapses to an NxHxH one.

Sharding: data-parallel over batch B across the 8 cores (1 batch per core).
The outputs (2 x 16 MB fp32 per core) dominate -> memory-bound.

Implementation notes:
  - All attention matmuls use float32r (TF32-like, 1 cycle/row vs 4 for
    fp32); the ~1e-4 rounding noise is attenuated through sigmoid to ~2e-5
    on global_pair.
  - Output tiles [j=128, h=256] = X[j,:] + X[i,:] are built exactly:
    PE broadcasts rows via ones-matmuls in f32r using a compensated pair
    (rowR = f32r(x), rowE = f32r(x - rowR); error ~1e-8), or GPSIMD
    partition_broadcast (bit-exact); DVE/ACT assemble into SBUF staging,
    8 tiles per 1 MB DMA store.
  - Variant mix per tile ('A' PE+ACT, 'B' PE+DVE-copy, 'C' PE-rows+DVE-add,
    'G' POOL-bcast+DVE-add) balances the four engines.
"""

import numpy as np

B, N, H, BIN = 8, 128, 256, 16
NCORES = 8
CPAD = 32        # c dim padded 16 -> 32 so transposed blocks land 32-aligned
IG = 4           # i's per binary-transpose group (4 * 32 = 128)
STAGE_I = 8      # output tiles per staged 1MB DMA store
PROLOG = 16      # local_pair tiles emitted before attention work starts

# variant per tile (A: PE-psum+ACT-copy, C: PE-row-psum+DVE-add,
# G: POOL-bcast+DVE-add). G positions are arithmetic (i%16 = 2+3k) so all
# G rows of a phase load with a single strided DMA.
VAR_PAT = "ACGACGCAGACGCAGC"


def variant_of(i):
    return VAR_PAT[i % 16]

_cache = {}


def _body(tc, io, reps=1):
    import concourse.bass as bass
    import concourse.mybir as mybir
    from concourse.masks import make_identity
    from contextlib import ExitStack, nullcontext

    nc = tc.nc
    ts = bass.ts
    f32 = mybir.dt.float32
    f32r = mybir.dt.float32r
    Relu = mybir.ActivationFunctionType.Relu
    Sigmoid = mybir.ActivationFunctionType.Sigmoid

    local_d, binary_d, wap_d, bap_d, wbin_d, bbin_d, watt_d, batt_d, lp_d, gp_d = io

    ctx = ExitStack()
    with ctx:
        persist = ctx.enter_context(tc.tile_pool(name="persist", bufs=1))
        binTp = ctx.enter_context(tc.tile_pool(name="binTp", bufs=6))
        att2p = ctx.enter_context(tc.tile_pool(name="att2p", bufs=3))
        stagep = ctx.enter_context(tc.tile_pool(name="stagep", bufs=2))
        bcastp = ctx.enter_context(tc.tile_pool(name="bcastp", bufs=3))
        prep = ctx.enter_context(tc.tile_pool(name="prep", bufs=2, space="PSUM"))
        outpp = ctx.enter_context(tc.tile_pool(name="outpp", bufs=4, space="PSUM"))
        dramp = ctx.enter_context(tc.tile_pool(name="dramp", bufs=1, space="DRAM"))

        # timing builds wrap the whole body in a device-side loop
        loop = tc.For_i(0, reps, 1) if reps > 1 else nullcontext()
        ctx.enter_context(loop)

        # ---------------- persistent setup ----------------
        identity = persist.tile([128, 128], f32, tag="identity")
        make_identity(nc, identity)
        identR = persist.tile([128, 128], f32r, tag="identR")
        nc.vector.tensor_copy(out=identR, in_=identity)
        onesF = persist.tile([128, 128], f32, tag="onesF")
        nc.gpsimd.memset(onesF, 1.0)
        onesT = persist.tile([128, 128], f32r, tag="onesT")
        nc.vector.tensor_copy(out=onesT, in_=onesF)

        localSb = persist.tile([N, H], f32, tag="localSb")
        nc.sync.dma_start(out=localSb, in_=local_d)

        # f32r weights (cast during SWDGE load)
        wap0 = persist.tile([128, H], f32r, tag="wap0")
        nc.gpsimd.dma_start(out=wap0, in_=wap_d[0:128])
        wap1 = persist.tile([128, H], f32r, tag="wap1")
        nc.gpsimd.dma_start(out=wap1, in_=wap_d[128:256])

        biasA = persist.tile([1, H], f32, tag="biasA")
        nc.sync.dma_start(out=biasA, in_=bap_d.unsqueeze(0))
        biasB = persist.tile([1, H], f32, tag="biasB")
        nc.sync.dma_start(out=biasB, in_=bbin_d.unsqueeze(0))
        biasRow = persist.tile([1, H], f32r, tag="biasRow")
        nc.vector.tensor_add(out=biasRow, in0=biasA, in1=biasB)

        wbinR = persist.tile([16, H], f32r, tag="wbinR")
        nc.gpsimd.dma_start(out=wbinR, in_=wbin_d)

        wattRow = persist.tile([1, H], f32, tag="wattRow")
        nc.sync.dma_start(out=wattRow, in_=watt_d.rearrange("k o -> o k"))
        battRow = persist.tile([1, 1], f32, tag="battRow")
        nc.sync.dma_start(out=battRow, in_=batt_d.unsqueeze(0))

        # Wx4: W_binary + bias row replicated at partitions {0,32,64,96}
        wx4 = persist.tile([128, H], f32r, tag="wx4")
        for m in range(4):
            nc.sync.dma_start(out=wx4[32 * m : 32 * m + 16, :], in_=wbinR)
            nc.sync.dma_start(out=wx4[32 * m + 16 : 32 * m + 17, :], in_=biasRow)

        # broadcast W_att across partitions; b_att as a [128,1] column
        wattB = persist.tile([128, H], f32, tag="wattB")
        battCol = persist.tile([128, 1], f32, tag="battCol")
        nc.gpsimd.partition_broadcast(wattB, wattRow)
        nc.gpsimd.partition_broadcast(battCol, battRow)

        # localT = local^T (f32r), then projW = local @ W_apair (f32r)
        localT = persist.tile([128, H], f32r, tag="localT")
        for hb in range(2):
            tp = outpp.tile([128, H], f32, tag="outp")
            nc.tensor.transpose(tp[:, 0:128], localSb[:, ts(hb, 128)], identity)
            nc.scalar.copy(out=localT[:, ts(hb, 128)], in_=tp[:, 0:128])
        pp = outpp.tile([128, H], f32, tag="outp")
        nc.tensor.matmul(pp, lhsT=localT[:, 0:128], rhs=wap0, start=True, stop=False)
        nc.tensor.matmul(pp, lhsT=localT[:, 128:256], rhs=wap1, start=False, stop=True)
        projW = persist.tile([128, H], f32r, tag="projW")
        nc.scalar.copy(out=projW, in_=pp)

        projDram = dramp.tile([N, H], f32r, tag="projDram")
        nc.sync.dma_start(out=projDram, in_=projW)
        projFlat = persist.tile([97, 32 * H], f32r, tag="projFlat")
        pf4 = projDram.rearrange("(a x) h -> a (x h)", a=4)
        for q in range(4):
            nc.sync.dma_start(out=projFlat[32 * q : 32 * q + 1, :], in_=pf4[q : q + 1])

        # compensated f32r split of X for exact PE broadcasts:
        #   XR = f32r(X), XE = f32r(X - XR);  XR + XE == X to ~1e-8
        # flatX holds exact f32 rows (for the G-variant POOL broadcast).
        xR = persist.tile([N, H], f32r, tag="xR")
        xE = persist.tile([N, H], f32r, tag="xE")
        flatR = persist.tile([97, 32 * H], f32r, tag="flatR")
        flatE = persist.tile([97, 32 * H], f32r, tag="flatE")
        xRDram = dramp.tile([N, H], f32r, tag="xRDram")
        xEDram = dramp.tile([N, H], f32r, tag="xEDram")

        def split_x(xSb):
            """fill xR/xE from xSb and bounce rows into flatR/flatE"""
            nc.vector.tensor_copy(out=xR, in_=xSb)
            nc.vector.tensor_sub(out=xE, in0=xSb, in1=xR.bitcast(f32))
            nc.sync.dma_start(out=xRDram, in_=xR)
            nc.sync.dma_start(out=xEDram, in_=xE)
            r4 = xRDram.rearrange("(a x) h -> a (x h)", a=4)
            e4 = xEDram.rearrange("(a x) h -> a (x h)", a=4)
            for q in range(4):
                nc.sync.dma_start(out=flatR[32 * q : 32 * q + 1, :], in_=r4[q : q + 1])
                nc.sync.dma_start(out=flatE[32 * q : 32 * q + 1, :], in_=e4[q : q + 1])

        split_x(localSb)

        # binp[j, (i, c32)]: c 0..15 = binary[., i, j, .], c16 = 1.0 (bias lane)
        binp = persist.tile([128, N * CPAD], f32, tag="binp")
        nc.gpsimd.memset(binp, 0.0)
        binp3 = binp.rearrange("p (i c) -> p i c", c=CPAD)
        nc.gpsimd.memset(binp3[:, :, 16:17], 1.0)
        for q in range(4):
            nc.sync.dma_start(
                out=binp3[:, ts(q, 32), 0:BIN],
                in_=binary_d[ts(q, 32)].rearrange("i j c -> j i c"),
            )

        logits = persist.tile([128, N], f32, tag="logits")
        ttrS = persist.tile([128, H], f32, tag="ttrS")
        binT = {}

        # ---------------- helpers ----------------
        def row_rhs(flat, i, width):
            q, r = divmod(i, 32)
            return flat[32 * q : 32 * q + 1, r * H : r * H + width]

        def row_lhsT(i):
            q = i // 32
            return onesT[32 * q : 32 * q + 1, :]

        def row_tp(i):
            return (32 * (i // 32), 0)

        gRows = persist.tile([1, 40 * H], f32, tag="gRows")

        def load_g_rows(x_dram):
            # rows i%16 in {2,5,8,11,14}: offset 2*H, strides (16*H, 3*H)
            src = x_dram.rearrange("n h -> (n h)")
            src3 = bass.AP(src.tensor, src.offset + 2 * H,
                           [[16 * H, 8], [3 * H, 5], [1, H]])
            nc.sync.dma_start(out=gRows.rearrange("o (a b h) -> o a b h", a=8, b=5),
                              in_=src3.unsqueeze(0))

        def g_row(i):
            q, r = divmod(i, 16)
            g = q * 5 + (r - 2) // 3
            return gRows[0:1, g * H : (g + 1) * H]

        def out_tile(i, xSb, x_dram, stage, variant):
            dst = stage[:, ts(i % STAGE_I, H)]
            if variant in ("A", "B"):
                po = outpp.tile([128, H], f32, tag="outp")
                nc.tensor.matmul(po, lhsT=identR, rhs=xR, start=True, stop=False)
                nc.tensor.matmul(po, lhsT=identR, rhs=xE, start=False, stop=False)
                nc.tensor.matmul(po, lhsT=row_lhsT(i), rhs=row_rhs(flatR, i, H),
                                 start=False, stop=False, tile_position=row_tp(i))
                nc.tensor.matmul(po, lhsT=row_lhsT(i), rhs=row_rhs(flatE, i, H),
                                 start=False, stop=True, tile_position=row_tp(i))
                if variant == "A":
                    nc.scalar.copy(out=dst, in_=po)
                else:
                    nc.vector.tensor_copy(out=dst, in_=po)
            elif variant == "C":
                po = outpp.tile([128, H], f32, tag="outp")
                nc.tensor.matmul(po, lhsT=row_lhsT(i), rhs=row_rhs(flatR, i, H),
                                 start=True, stop=False, tile_position=row_tp(i))
                nc.tensor.matmul(po, lhsT=row_lhsT(i), rhs=row_rhs(flatE, i, H),
                                 start=False, stop=True, tile_position=row_tp(i))
                nc.vector.tensor_add(out=dst, in0=xSb, in1=po)
            else:  # 'G': bit-exact POOL broadcast + DVE add
                bt = bcastp.tile([128, H], f32, tag="bc")
                nc.gpsimd.partition_broadcast(bt, g_row(i))
                nc.vector.tensor_add(out=dst, in0=xSb, in1=bt)

        def flush_stage(i0, stage, dram_out):
            nc.sync.dma_start(
                out=dram_out[i0 : i0 + STAGE_I].rearrange("i j h -> j i h"),
                in_=stage.rearrange("p (i h) -> p i h", h=H),
            )

        def attn_step(i):
            g, il = divmod(i, IG)
            if il == 0:  # transpose this binary group: [j,(i4,c32)] -> [(i4,c32),j]
                tp = outpp.tile([128, H], f32, tag="outp")
                nc.tensor.transpose(tp[:, 0:128], binp[:, ts(g, 128)], identity)
                bt = binTp.tile([128, 128], f32r, tag="binT")
                nc.scalar.copy(out=bt, in_=tp[:, 0:128])
                binT[g] = bt
            if i % 2 == 1:
                return
            pre = prep.tile([128, 2 * H], f32, tag="pre")
            nc.tensor.matmul(pre, lhsT=row_lhsT(i), rhs=row_rhs(projFlat, i, 2 * H),
                             start=True, stop=False, tile_position=row_tp(i))
            for m in range(2):
                ii = i + m
                gg, iil = divmod(ii, IG)
                nc.tensor.matmul(pre[:, ts(m, H)], lhsT=identR, rhs=projW,
                                 start=False, stop=False)
                nc.tensor.matmul(
                    pre[:, ts(m, H)],
                    lhsT=binT[gg][32 * iil : 32 * iil + 17, :],
                    rhs=wx4[32 * iil : 32 * iil + 17, :],
                    start=False, stop=(m == 1), tile_position=(32 * iil, 0),
                )
            a2 = att2p.tile([128, 2 * H], f32, tag="att2")
            nc.scalar.activation(out=a2, in_=pre, func=Relu)
            for m in range(2):
                nc.vector.affine_mul_reduce(
                    out=ttrS, accum_out=logits[:, i + m : i + m + 1],
                    in0=a2[:, ts(m, H)], in1=wattB, scale=1.0, bias=0.0,
                )

        # ---------------- phase 1: local_pair + attention ----------------
        stage = None
        load_g_rows(local_d)
        for i in range(N):
            if i % STAGE_I == 0:
                stage = stagep.tile([128, STAGE_I * H], f32, tag="stage")
            out_tile(i, localSb, local_d, stage, variant_of(i))
            if i % STAGE_I == STAGE_I - 1:
                flush_stage(i - STAGE_I + 1, stage, lp_d)
            if i >= PROLOG:
                attn_step(i - PROLOG)
        for i in range(N - PROLOG, N):
            attn_step(i)

        # ---------------- scores -> glob ----------------
        scoreT = persist.tile([128, N], f32, tag="scoreT")
        nc.scalar.activation(out=scoreT, in_=logits, func=Sigmoid, bias=battCol)
        pg = outpp.tile([128, H], f32, tag="outp")
        nc.tensor.matmul(pg, lhsT=scoreT, rhs=localSb, start=True, stop=True)
        globSb = persist.tile([128, H], f32, tag="globSb")
        nc.vector.tensor_copy(out=globSb, in_=pg)
        globDram = dramp.tile([N, H], f32, tag="globDram")
        nc.sync.dma_start(out=globDram, in_=globSb)
        split_x(globSb)
        load_g_rows(globDram)

        # ---------------- phase 2: global_pair ----------------
        for i in range(N):
            if i % STAGE_I == 0:
                stage = stagep.tile([128, STAGE_I * H], f32, tag="stage")
            out_tile(i, globSb, globDram, stage, variant_of(i))
            if i % STAGE_I == STAGE_I - 1:
                flush_stage(i - STAGE_I + 1, stage, gp_d)


def _build(reps=1):
    import concourse.bass as bass  # noqa: F401
    from concourse import bacc
    import concourse.mybir as mybir
    import concourse.tile as tile

    f32 = mybir.dt.float32
    nc = bacc.Bacc(
        "TRN2",
        target_bir_lowering=False,
        debug=False,
        enable_asserts=False,
        num_devices=NCORES,
    )
    io = (
        nc.dram_tensor("local", [N, H], f32, kind="ExternalInput").ap(),
        nc.dram_tensor("binary", [N, N, BIN], f32, kind="ExternalInput").ap(),
        nc.dram_tensor("w_apair", [H, H], f32, kind="ExternalInput").ap(),
        nc.dram_tensor("b_apair", [H], f32, kind="ExternalInput").ap(),
        nc.dram_tensor("w_binary", [BIN, H], f32, kind="ExternalInput").ap(),
        nc.dram_tensor("b_binary", [H], f32, kind="ExternalInput").ap(),
        nc.dram_tensor("w_att", [H, 1], f32, kind="ExternalInput").ap(),
        nc.dram_tensor("b_att", [1], f32, kind="ExternalInput").ap(),
        nc.dram_tensor("out_lp", [N, N, H], f32, kind="ExternalOutput").ap(),
        nc.dram_tensor("out_gp", [N, N, H], f32, kind="ExternalOutput").ap(),
    )
    with tile.TileContext(nc) as tc:
        _body(tc, io, reps=reps)
    nc.compile()
    return nc


def _get_nc():
    if "nc" not in _cache:
        _cache["nc"] = _build()
    return _cache["nc"]


def _run(inputs, trace=False):
    from concourse.bass_utils import run_bass_kernel_spmd

    nc = _get_nc()
    f = lambda x: np.ascontiguousarray(np.asarray(x), dtype=np.float32)
    shared = {
        "w_apair": f(inputs["W_apair"]),
        "b_apair": f(inputs["b_apair"]),
        "w_binary": f(inputs["W_binary"]),
        "b_binary": f(inputs["b_binary"]),
        "w_att": f(inputs["W_att"]),
        "b_att": f(inputs["b_att"]),
    }
    local = f(inputs["local_feats"])
    binary = f(inputs["binary_feats"])
    in_maps = [
        {"local": local[c], "binary": binary[c], **shared} for c in range(NCORES)
    ]
    res = run_bass_kernel_spmd(
        nc, in_maps, core_ids=list(range(NCORES)), trace=trace
    )
    lp = np.stack([r["out_lp"] for r in res.results])
    gp = np.stack([r["out_gp"] for r in res.results])
    return (lp, gp), res


def kernel(**inputs):
    out, _ = _run(inputs, trace=False)
    return out


# revision 35
# speedup vs baseline: 2.4069x; 1.0065x over previous
"""Trainium2 Bass kernel for nn_Attention_54580444397738 (gnn_message_passing).

Math per batch b (B=8, N=128, H=256, C=16):
  proj         = local @ W_apair                                     [N, H]
  pre[i,j,:]   = proj[i,:] + proj[j,:] + binary[i,j,:] @ W_binary
                 + b_apair + b_binary                                [N, N, H]
  score[i,j]   = sigmoid(relu(pre[i,j,:]) . W_att + b_att)           [N, N]
  glob         = score @ local                                       [N, H]
  local_pair [i,j,:] = local[i,:] + local[j,:]                       (output 1)
  global_pair[i,j,:] = glob[i,:]  + glob[j,:]                        (output 2)

Key algebraic simplification: einsum("bijh,hk->bijk", local_pair, W_apair)
= proj[i,:] + proj[j,:], so the N^2xHxH matmul collapses to an NxHxH one.

Sharding: data-parallel over batch B across the 8 cores (1 batch per core).
The outputs (2 x 16 MB fp32 per core) dominate -> memory-bound.

Implementation notes:
  - All attention matmuls use float32r (TF32-like, 1 cycle/row vs 4 for
    fp32); the ~1e-4 rounding noise is attenuated through sigmoid to ~2e-5
    on global_pair.
  - Output tiles [j=128, h=256] = X[j,:] + X[i,:] are built exactly:
    PE broadcasts rows via ones-matmuls in f32r using a compensated pair
    (rowR = f32r(x), rowE = f32r(x - rowR); error ~1e-8), or GPSIMD
    partition_broadcast (bit-exact); DVE/ACT assemble into SBUF staging,
    8 tiles per 1 MB DMA store.
  - Variant mix per tile ('A' PE+ACT, 'B' PE+DVE-copy, 'C' PE-rows+DVE-add,
    'G' POOL-bcast+DVE-add) balances the four engines.
"""

import numpy as np

B, N, H, BIN = 8, 128, 256, 16
NCORES = 8
CPAD = 32        # c dim padded 16 -> 32 so transposed blocks land 32-aligned
IG = 4           # i's per binary-transpose group (4 * 32 = 128)
STAGE_I = 8      # output tiles per staged 1MB DMA store
PROLOG = 16      # local_pair tiles emitted before attention work starts

# variant per tile (A: PE-psum+ACT-copy, C: PE-row-psum+DVE-add,
# G: POOL-bcast+DVE-add). G positions are arithmetic (i%16 = 2+3k) so all
# G rows of a phase load with a single strided DMA.
VAR_PAT = "DdGDdGDdGDdGDdGC"
SKIP_ATTN = False  # probe knob: drop attention/score work (wrong gp values)


def variant_of(i):
    return VAR_PAT[i % 16]

_cache = {}


def _body(tc, io, reps=1):
    import concourse.bass as bass
    import concourse.mybir as mybir
    from concourse.masks import make_identity
    from contextlib import ExitStack, nullcontext

    nc = tc.nc
    ts = bass.ts
    f32 = mybir.dt.float32
    f32r = mybir.dt.float32r
    Relu = mybir.ActivationFunctionType.Relu
    Sigmoid = mybir.ActivationFunctionType.Sigmoid

    local_d, binary_d, wap_d, bap_d, wbin_d, bbin_d, watt_d, batt_d, lp_d, gp_d = io

    ctx = ExitStack()
    with ctx:
        persist = ctx.enter_context(tc.tile_pool(name="persist", bufs=1))
        binTp = ctx.enter_context(tc.tile_pool(name="binTp", bufs=6))
        att2p = ctx.enter_context(tc.tile_pool(name="att2p", bufs=4))
        stagep = ctx.enter_context(tc.tile_pool(name="stagep", bufs=2))
        bcastp = ctx.enter_context(tc.tile_pool(name="bcastp", bufs=3))
        prep = ctx.enter_context(tc.tile_pool(name="prep", bufs=3, space="PSUM"))
        outpp = ctx.enter_context(tc.tile_pool(name="outpp", bufs=4, space="PSUM"))
        dramp = ctx.enter_context(tc.tile_pool(name="dramp", bufs=1, space="DRAM"))

        # timing builds wrap the whole body in a device-side loop
        loop = tc.For_i(0, reps, 1) if reps > 1 else nullcontext()
        ctx.enter_context(loop)

        # ---------------- persistent setup ----------------
        identity = persist.tile([128, 128], f32, tag="identity")
        make_identity(nc, identity)
        identR = persist.tile([128, 128], f32r, tag="identR")
        nc.vector.tensor_copy(out=identR, in_=identity)
        onesF = persist.tile([128, 128], f32, tag="onesF")
        nc.gpsimd.memset(onesF, 1.0)
        onesT = persist.tile([128, 128], f32r, tag="onesT")
        nc.vector.tensor_copy(out=onesT, in_=onesF)

        localSb = persist.tile([N, H], f32, tag="localSb")
        nc.sync.dma_start(out=localSb, in_=local_d)

        # f32r weights (cast during SWDGE load)
        wap0 = persist.tile([128, H], f32r, tag="wap0")
        nc.gpsimd.dma_start(out=wap0, in_=wap_d[0:128])
        wap1 = persist.tile([128, H], f32r, tag="wap1")
        nc.gpsimd.dma_start(out=wap1, in_=wap_d[128:256])

        biasA = persist.tile([1, H], f32, tag="biasA")
        nc.sync.dma_start(out=biasA, in_=bap_d.unsqueeze(0))
        biasB = persist.tile([1, H], f32, tag="biasB")
        nc.sync.dma_start(out=biasB, in_=bbin_d.unsqueeze(0))
        biasRow = persist.tile([1, H], f32r, tag="biasRow")
        nc.vector.tensor_add(out=biasRow, in0=biasA, in1=biasB)

        wbinR = persist.tile([16, H], f32r, tag="wbinR")
        nc.gpsimd.dma_start(out=wbinR, in_=wbin_d)

        wattRow = persist.tile([1, H], f32, tag="wattRow")
        nc.sync.dma_start(out=wattRow, in_=watt_d.rearrange("k o -> o k"))
        battRow = persist.tile([1, 1], f32, tag="battRow")
        nc.sync.dma_start(out=battRow, in_=batt_d.unsqueeze(0))

        # Wx4: W_binary + bias row replicated at partitions {0,32,64,96}
        wx4 = persist.tile([128, H], f32r, tag="wx4")
        for m in range(4):
            nc.sync.dma_start(out=wx4[32 * m : 32 * m + 16, :], in_=wbinR)
            nc.sync.dma_start(out=wx4[32 * m + 16 : 32 * m + 17, :], in_=biasRow)

        # broadcast W_att across partitions; b_att as a [128,1] column
        wattB = persist.tile([128, H], f32, tag="wattB")
        battCol = persist.tile([128, 1], f32, tag="battCol")
        nc.gpsimd.partition_broadcast(wattB, wattRow)
        nc.gpsimd.partition_broadcast(battCol, battRow)

        # localT = local^T (f32r), then projW = local @ W_apair (f32r)
        localT = persist.tile([128, H], f32r, tag="localT")
        for hb in range(2):
            tp = outpp.tile([128, H], f32, tag="outp")
            nc.tensor.transpose(tp[:, 0:128], localSb[:, ts(hb, 128)], identity)
            nc.scalar.copy(out=localT[:, ts(hb, 128)], in_=tp[:, 0:128])
        pp = outpp.tile([128, H], f32, tag="outp")
        nc.tensor.matmul(pp, lhsT=localT[:, 0:128], rhs=wap0, start=True, stop=False)
        nc.tensor.matmul(pp, lhsT=localT[:, 128:256], rhs=wap1, start=False, stop=True)
        projW = persist.tile([128, H], f32r, tag="projW")
        nc.scalar.copy(out=projW, in_=pp)

        projDram = dramp.tile([N, H], f32r, tag="projDram")
        nc.sync.dma_start(out=projDram, in_=projW)
        projFlat = persist.tile([97, 32 * H], f32r, tag="projFlat")
        pf4 = projDram.rearrange("(a x) h -> a (x h)", a=4)
        for q in range(4):
            nc.sync.dma_start(out=projFlat[32 * q : 32 * q + 1, :], in_=pf4[q : q + 1])

        # compensated f32r split of X for exact PE broadcasts:
        #   XR = f32r(X), XE = f32r(X - XR);  XR + XE == X to ~1e-8
        # flatX holds exact f32 rows (for the G-variant POOL broadcast).
        xR = persist.tile([N, H], f32r, tag="xR")
        xE = persist.tile([N, H], f32r, tag="xE")
        flatR = persist.tile([97, 32 * H], f32r, tag="flatR")
        flatE = persist.tile([97, 32 * H], f32r, tag="flatE")
        xRDram = dramp.tile([N, H], f32r, tag="xRDram")
        xEDram = dramp.tile([N, H], f32r, tag="xEDram")

        def split_x(xSb):
            """fill xR/xE from xSb and bounce rows into flatR/flatE"""
            nc.vector.tensor_copy(out=xR, in_=xSb)
            nc.vector.tensor_sub(out=xE, in0=xSb, in1=xR.bitcast(f32))
            nc.sync.dma_start(out=xRDram, in_=xR)
            nc.sync.dma_start(out=xEDram, in_=xE)
            r4 = xRDram.rearrange("(a x) h -> a (x h)", a=4)
            e4 = xEDram.rearrange("(a x) h -> a (x h)", a=4)
            for q in range(4):
                nc.sync.dma_start(out=flatR[32 * q : 32 * q + 1, :], in_=r4[q : q + 1])
                nc.sync.dma_start(out=flatE[32 * q : 32 * q + 1, :], in_=e4[q : q + 1])

        split_x(localSb)

        # binp[j, (i, c32)]: c 0..15 = binary[., i, j, .], c16 = 1.0 (bias lane)
        binp = persist.tile([128, N * CPAD], f32, tag="binp")
        nc.gpsimd.memset(binp, 0.0)
        binp3 = binp.rearrange("p (i c) -> p i c", c=CPAD)
        nc.gpsimd.memset(binp3[:, :, 16:17], 1.0)
        for q in range(4):
            nc.sync.dma_start(
                out=binp3[:, ts(q, 32), 0:BIN],
                in_=binary_d[ts(q, 32)].rearrange("i j c -> j i c"),
            )

        logits = persist.tile([128, N], f32, tag="logits")
        ttrS = persist.tile([128, H], f32, tag="ttrS")
        binT = {}

        # ---------------- helpers ----------------
        def row_rhs(flat, i, width):
            q, r = divmod(i, 32)
            return flat[32 * q : 32 * q + 1, r * H : r * H + width]

        def row_lhsT(i):
            q = i // 32
            return onesT[32 * q : 32 * q + 1, :]

        def row_tp(i):
            return (32 * (i // 32), 0)

        gRows = persist.tile([1, 40 * H], f32, tag="gRows")

        def load_g_rows(x_dram):
            # rows i%16 in {2,5,8,11,14}: offset 2*H, strides (16*H, 3*H)
            src = x_dram.rearrange("n h -> (n h)")
            src3 = bass.AP(src.tensor, src.offset + 2 * H,
                           [[16 * H, 8], [3 * H, 5], [1, H]])
            nc.sync.dma_start(out=gRows.rearrange("o (a b h) -> o a b h", a=8, b=5),
                              in_=src3.unsqueeze(0))

        def g_row(i):
            q, r = divmod(i, 16)
            g = q * 5 + (r - 2) // 3
            return gRows[0:1, g * H : (g + 1) * H]

        def out_tile_pair(i, xSb, stage):
            # tiles i, i+1 in one psum bank: rows via N=512 f32r pair-MMs,
            # one fused DVE add with free-broadcast of the X tile.
            dst = stage[:, (i % STAGE_I) * H : (i % STAGE_I) * H + 2 * H]
            po = prep.tile([128, 2 * H], f32, tag="pre")
            nc.tensor.matmul(po, lhsT=row_lhsT(i), rhs=row_rhs(flatR, i, 2 * H),
                             start=True, stop=False, tile_position=row_tp(i))
            nc.tensor.matmul(po, lhsT=row_lhsT(i), rhs=row_rhs(flatE, i, 2 * H),
                             start=False, stop=True, tile_position=row_tp(i))
            nc.vector.tensor_add(
                out=dst, in0=xSb.unsqueeze(1).broadcast_to([128, 2, H]), in1=po)

        def out_tile(i, xSb, x_dram, stage, variant):
            dst = stage[:, ts(i % STAGE_I, H)]
            if variant in ("A", "B"):
                po = outpp.tile([128, H], f32, tag="outp")
                nc.tensor.matmul(po, lhsT=identR, rhs=xR, start=True, stop=False)
                nc.tensor.matmul(po, lhsT=identR, rhs=xE, start=False, stop=False)
                nc.tensor.matmul(po, lhsT=row_lhsT(i), rhs=row_rhs(flatR, i, H),
                                 start=False, stop=False, tile_position=row_tp(i))
                nc.tensor.matmul(po, lhsT=row_lhsT(i), rhs=row_rhs(flatE, i, H),
                                 start=False, stop=True, tile_position=row_tp(i))
                if variant == "A":
                    nc.scalar.copy(out=dst, in_=po)
                else:
                    nc.vector.tensor_copy(out=dst, in_=po)
            elif variant == "C":
                po = outpp.tile([128, H], f32, tag="outp")
                nc.tensor.matmul(po, lhsT=row_lhsT(i), rhs=row_rhs(flatR, i, H),
                                 start=True, stop=False, tile_position=row_tp(i))
                nc.tensor.matmul(po, lhsT=row_lhsT(i), rhs=row_rhs(flatE, i, H),
                                 start=False, stop=True, tile_position=row_tp(i))
                nc.vector.tensor_add(out=dst, in0=xSb, in1=po)
            else:  # 'G': bit-exact POOL broadcast + DVE add
                bt = bcastp.tile([128, H], f32, tag="bc")
                nc.gpsimd.partition_broadcast(bt, g_row(i))
                nc.vector.tensor_add(out=dst, in0=xSb, in1=bt)

        def flush_stage(i0, stage, dram_out):
            nc.sync.dma_start(
                out=dram_out[i0 : i0 + STAGE_I].rearrange("i j h -> j i h"),
                in_=stage.rearrange("p (i h) -> p i h", h=H),
            )

        def attn_step(i):
            g, il = divmod(i, IG)
            if il == 0:  # transpose this binary group: [j,(i4,c32)] -> [(i4,c32),j]
                tp = outpp.tile([128, H], f32, tag="outp")
                nc.tensor.transpose(tp[:, 0:128], binp[:, ts(g, 128)], identity)
                bt = binTp.tile([128, 128], f32r, tag="binT")
                nc.scalar.copy(out=bt, in_=tp[:, 0:128])
                binT[g] = bt
            if i % 2 == 1:
                return
            pre = prep.tile([128, 2 * H], f32, tag="pre")
            nc.tensor.matmul(pre, lhsT=row_lhsT(i), rhs=row_rhs(projFlat, i, 2 * H),
                             start=True, stop=False, tile_position=row_tp(i))
            for m in range(2):
                ii = i + m
                gg, iil = divmod(ii, IG)
                nc.tensor.matmul(pre[:, ts(m, H)], lhsT=identR, rhs=projW,
                                 start=False, stop=False)
                nc.tensor.matmul(
                    pre[:, ts(m, H)],
                    lhsT=binT[gg][32 * iil : 32 * iil + 17, :],
                    rhs=wx4[32 * iil : 32 * iil + 17, :],
                    start=False, stop=(m == 1), tile_position=(32 * iil, 0),
                )
            a2 = att2p.tile([128, 2 * H], f32, tag="att2")
            nc.scalar.activation(out=a2, in_=pre, func=Relu)
            for m in range(2):
                nc.vector.affine_mul_reduce(
                    out=ttrS, accum_out=logits[:, i + m : i + m + 1],
                    in0=a2[:, ts(m, H)], in1=wattB, scale=1.0, bias=0.0,
                )

        # ---------------- phase 1: local_pair + attention ----------------
        def out_phase(xSb, x_dram, dram_out, with_attn):
            stage = None
            i = 0
            attn_at = 0

            def attn_tick(limit):
                nonlocal attn_at
                if with_attn:
                    while attn_at < min(limit, N):
                        attn_step(attn_at)
                        attn_at += 1

            while i < N:
                if i % STAGE_I == 0:
                    stage = stagep.tile([128, STAGE_I * H], f32, tag="stage")
                v = variant_of(i)
                if v == "D":
                    out_tile_pair(i, xSb, stage)
                    step = 2
                else:
                    out_tile(i, xSb, x_dram, stage, v)
                    step = 1
                for k in range(step):
                    if (i + k) % STAGE_I == STAGE_I - 1:
                        flush_stage(i + k - STAGE_I + 1, stage, dram_out)
                i += step
                attn_tick(i - PROLOG)
            attn_tick(N)

        load_g_rows(local_d)
        out_phase(localSb, local_d, lp_d, with_attn=not SKIP_ATTN)

        # ---------------- scores -> glob ----------------
        scoreT = persist.tile([128, N], f32, tag="scoreT")
        globSb = persist.tile([128, H], f32, tag="globSb")
        if SKIP_ATTN:
            nc.vector.tensor_copy(out=globSb, in_=localSb)
        else:
            nc.scalar.activation(out=scoreT, in_=logits, func=Sigmoid, bias=battCol)
            pg = outpp.tile([128, H], f32, tag="outp")
            nc.tensor.matmul(pg, lhsT=scoreT, rhs=localSb, start=True, stop=True)
            nc.vector.tensor_copy(out=globSb, in_=pg)
        globDram = dramp.tile([N, H], f32, tag="globDram")
        nc.sync.dma_start(out=globDram, in_=globSb)
        split_x(globSb)
        load_g_rows(globDram)

        # ---------------- phase 2: global_pair ----------------
        out_phase(globSb, globDram, gp_d, with_attn=False)


def _build(reps=1):
    import concourse.bass as bass  # noqa: F401
    from concourse import bacc
    import concourse.mybir as mybir
    import concourse.tile as tile

    f32 = mybir.dt.float32
    nc = bacc.Bacc(
        "TRN2",
        target_bir_lowering=False,
        debug=False,
        enable_asserts=False,
        num_devices=NCORES,
    )
    io = (
        nc.dram_tensor("local", [N, H], f32, kind="ExternalInput").ap(),
        nc.dram_tensor("binary", [N, N, BIN], f32, kind="ExternalInput").ap(),
        nc.dram_tensor("w_apair", [H, H], f32, kind="ExternalInput").ap(),
        nc.dram_tensor("b_apair", [H], f32, kind="ExternalInput").ap(),
        nc.dram_tensor("w_binary", [BIN, H], f32, kind="ExternalInput").ap(),
        nc.dram_tensor("b_binary", [H], f32, kind="ExternalInput").ap(),
        nc.dram_tensor("w_att", [H, 1], f32, kind="ExternalInput").ap(),
        nc.dram_tensor("b_att", [1], f32, kind="ExternalInput").ap(),
        nc.dram_tensor("out_lp", [N, N, H], f32, kind="ExternalOutput").ap(),
        nc.dram_tensor("out_gp", [N, N, H], f32, kind="ExternalOutput").ap(),
    )
    with tile.TileContext(nc) as tc:
        _body(tc, io, reps=reps)
    nc.compile()
    return nc


def _get_nc():
    if "nc" not in _cache:
        _cache["nc"] = _build()
    return _cache["nc"]


def _run(inputs, trace=False):
    from concourse.bass_utils import run_bass_kernel_spmd

    nc = _get_nc()
    f = lambda x: np.ascontiguousarray(np.asarray(x), dtype=np.float32)
    shared = {
        "w_apair": f(inputs["W_apair"]),
        "b_apair": f(inputs["b_apair"]),
        "w_binary": f(inputs["W_binary"]),
        "b_binary": f(inputs["b_binary"]),
        "w_att": f(inputs["W_att"]),
        "b_att": f(inputs["b_att"]),
    }
    local = f(inputs["local_feats"])
    binary = f(inputs["binary_feats"])
    in_maps = [
        {"local": local[c], "binary": binary[c], **shared} for c in range(NCORES)
    ]
    res = run_bass_kernel_spmd(
        nc, in_maps, core_ids=list(range(NCORES)), trace=trace
    )
    lp = np.stack([r["out_lp"] for r in res.results])
    gp = np.stack([r["out_gp"] for r in res.results])
    return (lp, gp), res


def kernel(**inputs):
    out, _ = _run(inputs, trace=False)
    return out


# revision 36
# speedup vs baseline: 2.5296x; 1.0510x over previous
"""Trainium2 Bass kernel for nn_Attention_54580444397738 (gnn_message_passing).

Math per batch b (B=8, N=128, H=256, C=16):
  proj         = local @ W_apair                                     [N, H]
  pre[i,j,:]   = proj[i,:] + proj[j,:] + binary[i,j,:] @ W_binary
                 + b_apair + b_binary                                [N, N, H]
  score[i,j]   = sigmoid(relu(pre[i,j,:]) . W_att + b_att)           [N, N]
  glob         = score @ local                                       [N, H]
  local_pair [i,j,:] = local[i,:] + local[j,:]                       (output 1)
  global_pair[i,j,:] = glob[i,:]  + glob[j,:]                        (output 2)

Key algebraic simplification: einsum("bijh,hk->bijk", local_pair, W_apair)
= proj[i,:] + proj[j,:], so the N^2xHxH matmul collapses to an NxHxH one.

Sharding: data-parallel over batch B across the 8 cores (1 batch per core).
The outputs (2 x 16 MB fp32 per core) dominate -> memory-bound.

Implementation notes:
  - All attention matmuls use float32r (TF32-like, 1 cycle/row vs 4 for
    fp32); the ~1e-4 rounding noise is attenuated through sigmoid to ~2e-5
    on global_pair.
  - Output tiles [j=128, h=256] = X[j,:] + X[i,:] are built exactly:
    PE broadcasts rows via ones-matmuls in f32r using a compensated pair
    (rowR = f32r(x), rowE = f32r(x - rowR); error ~1e-8), or GPSIMD
    partition_broadcast (bit-exact); DVE/ACT assemble into SBUF staging,
    8 tiles per 1 MB DMA store.
  - Variant mix per tile ('A' PE+ACT, 'B' PE+DVE-copy, 'C' PE-rows+DVE-add,
    'G' POOL-bcast+DVE-add) balances the four engines.
"""

import numpy as np

B, N, H, BIN = 8, 128, 256, 16
NCORES = 8
CPAD = 32        # c dim padded 16 -> 32 so transposed blocks land 32-aligned
IG = 4           # i's per binary-transpose group (4 * 32 = 128)
STAGE_I = 8      # output tiles per staged 1MB DMA store
PROLOG = 16      # local_pair tiles emitted before attention work starts

# variant per tile (A: PE-psum+ACT-copy, C: PE-row-psum+DVE-add,
# G: POOL-bcast+DVE-add). G positions are arithmetic (i%16 = 2+3k) so all
# G rows of a phase load with a single strided DMA.
VAR_PAT = "DdGEeGDdGEeGDdGC"
SKIP_ATTN = False  # probe knob: drop attention/score work (wrong gp values)


def variant_of(i):
    return VAR_PAT[i % 16]

_cache = {}


def _body(tc, io, reps=1):
    import concourse.bass as bass
    import concourse.mybir as mybir
    from concourse.masks import make_identity
    from contextlib import ExitStack, nullcontext

    nc = tc.nc
    ts = bass.ts
    f32 = mybir.dt.float32
    f32r = mybir.dt.float32r
    Relu = mybir.ActivationFunctionType.Relu
    Sigmoid = mybir.ActivationFunctionType.Sigmoid

    local_d, binary_d, wap_d, bap_d, wbin_d, bbin_d, watt_d, batt_d, lp_d, gp_d = io

    ctx = ExitStack()
    with ctx:
        persist = ctx.enter_context(tc.tile_pool(name="persist", bufs=1))
        binTp = ctx.enter_context(tc.tile_pool(name="binTp", bufs=6))
        att2p = ctx.enter_context(tc.tile_pool(name="att2p", bufs=4))
        stagep = ctx.enter_context(tc.tile_pool(name="stagep", bufs=2))
        bcastp = ctx.enter_context(tc.tile_pool(name="bcastp", bufs=3))
        prep = ctx.enter_context(tc.tile_pool(name="prep", bufs=3, space="PSUM"))
        outpp = ctx.enter_context(tc.tile_pool(name="outpp", bufs=4, space="PSUM"))
        dramp = ctx.enter_context(tc.tile_pool(name="dramp", bufs=1, space="DRAM"))

        # timing builds wrap the whole body in a device-side loop
        loop = tc.For_i(0, reps, 1) if reps > 1 else nullcontext()
        ctx.enter_context(loop)

        # ---------------- persistent setup ----------------
        identity = persist.tile([128, 128], f32, tag="identity")
        make_identity(nc, identity)
        identR = persist.tile([128, 128], f32r, tag="identR")
        nc.vector.tensor_copy(out=identR, in_=identity)
        onesF = persist.tile([128, 128], f32, tag="onesF")
        nc.gpsimd.memset(onesF, 1.0)
        onesT = persist.tile([128, 128], f32r, tag="onesT")
        nc.vector.tensor_copy(out=onesT, in_=onesF)

        localSb = persist.tile([N, H], f32, tag="localSb")
        nc.sync.dma_start(out=localSb, in_=local_d)

        # f32r weights (cast during SWDGE load)
        wap0 = persist.tile([128, H], f32r, tag="wap0")
        nc.gpsimd.dma_start(out=wap0, in_=wap_d[0:128])
        wap1 = persist.tile([128, H], f32r, tag="wap1")
        nc.gpsimd.dma_start(out=wap1, in_=wap_d[128:256])

        biasA = persist.tile([1, H], f32, tag="biasA")
        nc.sync.dma_start(out=biasA, in_=bap_d.unsqueeze(0))
        biasB = persist.tile([1, H], f32, tag="biasB")
        nc.sync.dma_start(out=biasB, in_=bbin_d.unsqueeze(0))
        biasRow = persist.tile([1, H], f32r, tag="biasRow")
        nc.vector.tensor_add(out=biasRow, in0=biasA, in1=biasB)

        wbinR = persist.tile([16, H], f32r, tag="wbinR")
        nc.gpsimd.dma_start(out=wbinR, in_=wbin_d)

        wattRow = persist.tile([1, H], f32, tag="wattRow")
        nc.sync.dma_start(out=wattRow, in_=watt_d.rearrange("k o -> o k"))
        battRow = persist.tile([1, 1], f32, tag="battRow")
        nc.sync.dma_start(out=battRow, in_=batt_d.unsqueeze(0))

        # Wx4: W_binary + bias row replicated at partitions {0,32,64,96}
        wx4 = persist.tile([128, H], f32r, tag="wx4")
        for m in range(4):
            nc.sync.dma_start(out=wx4[32 * m : 32 * m + 16, :], in_=wbinR)
            nc.sync.dma_start(out=wx4[32 * m + 16 : 32 * m + 17, :], in_=biasRow)

        # broadcast W_att across partitions; b_att as a [128,1] column
        wattB = persist.tile([128, H], f32, tag="wattB")
        battCol = persist.tile([128, 1], f32, tag="battCol")
        nc.gpsimd.partition_broadcast(wattB, wattRow)
        nc.gpsimd.partition_broadcast(battCol, battRow)

        # localT = local^T (f32r), then projW = local @ W_apair (f32r)
        localT = persist.tile([128, H], f32r, tag="localT")
        for hb in range(2):
            tp = outpp.tile([128, H], f32, tag="outp")
            nc.tensor.transpose(tp[:, 0:128], localSb[:, ts(hb, 128)], identity)
            nc.scalar.copy(out=localT[:, ts(hb, 128)], in_=tp[:, 0:128])
        pp = outpp.tile([128, H], f32, tag="outp")
        nc.tensor.matmul(pp, lhsT=localT[:, 0:128], rhs=wap0, start=True, stop=False)
        nc.tensor.matmul(pp, lhsT=localT[:, 128:256], rhs=wap1, start=False, stop=True)
        projW = persist.tile([128, H], f32r, tag="projW")
        nc.scalar.copy(out=projW, in_=pp)

        projDram = dramp.tile([N, H], f32r, tag="projDram")
        nc.sync.dma_start(out=projDram, in_=projW)
        projFlat = persist.tile([97, 32 * H], f32r, tag="projFlat")
        pf4 = projDram.rearrange("(a x) h -> a (x h)", a=4)
        for q in range(4):
            nc.sync.dma_start(out=projFlat[32 * q : 32 * q + 1, :], in_=pf4[q : q + 1])

        # compensated f32r split of X for exact PE broadcasts:
        #   XR = f32r(X), XE = f32r(X - XR);  XR + XE == X to ~1e-8
        # flatX holds exact f32 rows (for the G-variant POOL broadcast).
        xR = persist.tile([N, H], f32r, tag="xR")
        xE = persist.tile([N, H], f32r, tag="xE")
        flatR = persist.tile([97, 32 * H], f32r, tag="flatR")
        flatE = persist.tile([97, 32 * H], f32r, tag="flatE")
        xRDram = dramp.tile([N, H], f32r, tag="xRDram")
        xEDram = dramp.tile([N, H], f32r, tag="xEDram")

        def split_x(xSb):
            """fill xR/xE from xSb and bounce rows into flatR/flatE"""
            nc.vector.tensor_copy(out=xR, in_=xSb)
            nc.vector.tensor_sub(out=xE, in0=xSb, in1=xR.bitcast(f32))
            nc.sync.dma_start(out=xRDram, in_=xR)
            nc.sync.dma_start(out=xEDram, in_=xE)
            r4 = xRDram.rearrange("(a x) h -> a (x h)", a=4)
            e4 = xEDram.rearrange("(a x) h -> a (x h)", a=4)
            for q in range(4):
                nc.sync.dma_start(out=flatR[32 * q : 32 * q + 1, :], in_=r4[q : q + 1])
                nc.sync.dma_start(out=flatE[32 * q : 32 * q + 1, :], in_=e4[q : q + 1])

        split_x(localSb)

        # binp[j, (i, c32)]: c 0..15 = binary[., i, j, .], c16 = 1.0 (bias lane)
        binp = persist.tile([128, N * CPAD], f32, tag="binp")
        nc.gpsimd.memset(binp, 0.0)
        binp3 = binp.rearrange("p (i c) -> p i c", c=CPAD)
        nc.gpsimd.memset(binp3[:, :, 16:17], 1.0)
        for q in range(4):
            nc.sync.dma_start(
                out=binp3[:, ts(q, 32), 0:BIN],
                in_=binary_d[ts(q, 32)].rearrange("i j c -> j i c"),
            )

        logits = persist.tile([128, N], f32, tag="logits")
        ttrS = persist.tile([128, H], f32, tag="ttrS")
        binT = {}

        # ---------------- helpers ----------------
        def row_rhs(flat, i, width):
            q, r = divmod(i, 32)
            return flat[32 * q : 32 * q + 1, r * H : r * H + width]

        def row_lhsT(i):
            q = i // 32
            return onesT[32 * q : 32 * q + 1, :]

        def row_tp(i):
            return (32 * (i // 32), 0)

        gRows = persist.tile([1, 40 * H], f32, tag="gRows")

        def load_g_rows(x_dram):
            # rows i%16 in {2,5,8,11,14}: offset 2*H, strides (16*H, 3*H)
            src = x_dram.rearrange("n h -> (n h)")
            src3 = bass.AP(src.tensor, src.offset + 2 * H,
                           [[16 * H, 8], [3 * H, 5], [1, H]])
            nc.sync.dma_start(out=gRows.rearrange("o (a b h) -> o a b h", a=8, b=5),
                              in_=src3.unsqueeze(0))

        def g_row(i):
            q, r = divmod(i, 16)
            g = q * 5 + (r - 2) // 3
            return gRows[0:1, g * H : (g + 1) * H]

        def out_tile_pair(i, xSb, stage):
            # tiles i, i+1 in one psum bank: rows via N=512 f32r pair-MMs,
            # one fused DVE add with free-broadcast of the X tile.
            dst = stage[:, (i % STAGE_I) * H : (i % STAGE_I) * H + 2 * H]
            po = prep.tile([128, 2 * H], f32, tag="pre")
            nc.tensor.matmul(po, lhsT=row_lhsT(i), rhs=row_rhs(flatR, i, 2 * H),
                             start=True, stop=False, tile_position=row_tp(i))
            nc.tensor.matmul(po, lhsT=row_lhsT(i), rhs=row_rhs(flatE, i, 2 * H),
                             start=False, stop=True, tile_position=row_tp(i))
            nc.vector.tensor_add(
                out=dst, in0=xSb.unsqueeze(1).broadcast_to([128, 2, H]), in1=po)

        def out_tile_pairA(i, xSb, stage):
            # tiles i, i+1 via full psum pair + ACT copy (broadcast-rhs MMs)
            dst = stage[:, (i % STAGE_I) * H : (i % STAGE_I) * H + 2 * H]
            po = prep.tile([128, 2 * H], f32, tag="pre")
            xr2 = xR.unsqueeze(1).broadcast_to([128, 2, H])
            xe2 = xE.unsqueeze(1).broadcast_to([128, 2, H])
            nc.tensor.matmul(po, lhsT=identR, rhs=xr2, start=True, stop=False)
            nc.tensor.matmul(po, lhsT=identR, rhs=xe2, start=False, stop=False)
            nc.tensor.matmul(po, lhsT=row_lhsT(i), rhs=row_rhs(flatR, i, 2 * H),
                             start=False, stop=False, tile_position=row_tp(i))
            nc.tensor.matmul(po, lhsT=row_lhsT(i), rhs=row_rhs(flatE, i, 2 * H),
                             start=False, stop=True, tile_position=row_tp(i))
            nc.scalar.copy(out=dst, in_=po)

        def out_tile(i, xSb, x_dram, stage, variant):
            dst = stage[:, ts(i % STAGE_I, H)]
            if variant in ("A", "B"):
                po = outpp.tile([128, H], f32, tag="outp")
                nc.tensor.matmul(po, lhsT=identR, rhs=xR, start=True, stop=False)
                nc.tensor.matmul(po, lhsT=identR, rhs=xE, start=False, stop=False)
                nc.tensor.matmul(po, lhsT=row_lhsT(i), rhs=row_rhs(flatR, i, H),
                                 start=False, stop=False, tile_position=row_tp(i))
                nc.tensor.matmul(po, lhsT=row_lhsT(i), rhs=row_rhs(flatE, i, H),
                                 start=False, stop=True, tile_position=row_tp(i))
                if variant == "A":
                    nc.scalar.copy(out=dst, in_=po)
                else:
                    nc.vector.tensor_copy(out=dst, in_=po)
            elif variant == "C":
                po = outpp.tile([128, H], f32, tag="outp")
                nc.tensor.matmul(po, lhsT=row_lhsT(i), rhs=row_rhs(flatR, i, H),
                                 start=True, stop=False, tile_position=row_tp(i))
                nc.tensor.matmul(po, lhsT=row_lhsT(i), rhs=row_rhs(flatE, i, H),
                                 start=False, stop=True, tile_position=row_tp(i))
                nc.vector.tensor_add(out=dst, in0=xSb, in1=po)
            else:  # 'G': bit-exact POOL broadcast + DVE add
                bt = bcastp.tile([128, H], f32, tag="bc")
                nc.gpsimd.partition_broadcast(bt, g_row(i))
                nc.vector.tensor_add(out=dst, in0=xSb, in1=bt)

        def flush_stage(i0, stage, dram_out):
            nc.sync.dma_start(
                out=dram_out[i0 : i0 + STAGE_I].rearrange("i j h -> j i h"),
                in_=stage.rearrange("p (i h) -> p i h", h=H),
            )

        def attn_step(i):
            g, il = divmod(i, IG)
            if il == 0:  # transpose this binary group: [j,(i4,c32)] -> [(i4,c32),j]
                tp = outpp.tile([128, H], f32, tag="outp")
                nc.tensor.transpose(tp[:, 0:128], binp[:, ts(g, 128)], identity)
                bt = binTp.tile([128, 128], f32r, tag="binT")
                nc.scalar.copy(out=bt, in_=tp[:, 0:128])
                binT[g] = bt
            if i % 2 == 1:
                return
            pre = prep.tile([128, 2 * H], f32, tag="pre")
            nc.tensor.matmul(pre, lhsT=row_lhsT(i), rhs=row_rhs(projFlat, i, 2 * H),
                             start=True, stop=False, tile_position=row_tp(i))
            for m in range(2):
                ii = i + m
                gg, iil = divmod(ii, IG)
                nc.tensor.matmul(pre[:, ts(m, H)], lhsT=identR, rhs=projW,
                                 start=False, stop=False)
                nc.tensor.matmul(
                    pre[:, ts(m, H)],
                    lhsT=binT[gg][32 * iil : 32 * iil + 17, :],
                    rhs=wx4[32 * iil : 32 * iil + 17, :],
                    start=False, stop=(m == 1), tile_position=(32 * iil, 0),
                )
            a2 = att2p.tile([128, 2 * H], f32, tag="att2")
            nc.scalar.activation(out=a2, in_=pre, func=Relu)
            for m in range(2):
                nc.vector.affine_mul_reduce(
                    out=ttrS, accum_out=logits[:, i + m : i + m + 1],
                    in0=a2[:, ts(m, H)], in1=wattB, scale=1.0, bias=0.0,
                )

        # ---------------- phase 1: local_pair + attention ----------------
        def out_phase(xSb, x_dram, dram_out, with_attn):
            stage = None
            i = 0
            attn_at = 0

            def attn_tick(limit):
                nonlocal attn_at
                if with_attn:
                    while attn_at < min(limit, N):
                        attn_step(attn_at)
                        attn_at += 1

            while i < N:
                if i % STAGE_I == 0:
                    stage = stagep.tile([128, STAGE_I * H], f32, tag="stage")
                v = variant_of(i)
                if v == "D":
                    out_tile_pair(i, xSb, stage)
                    step = 2
                elif v == "E":
                    out_tile_pairA(i, xSb, stage)
                    step = 2
                else:
                    out_tile(i, xSb, x_dram, stage, v)
                    step = 1
                for k in range(step):
                    if (i + k) % STAGE_I == STAGE_I - 1:
                        flush_stage(i + k - STAGE_I + 1, stage, dram_out)
                i += step
                attn_tick(i - PROLOG)
            attn_tick(N)

        load_g_rows(local_d)
        out_phase(localSb, local_d, lp_d, with_attn=not SKIP_ATTN)

        # ---------------- scores -> glob ----------------
        scoreT = persist.tile([128, N], f32, tag="scoreT")
        globSb = persist.tile([128, H], f32, tag="globSb")
        if SKIP_ATTN:
            nc.vector.tensor_copy(out=globSb, in_=localSb)
        else:
            nc.scalar.activation(out=scoreT, in_=logits, func=Sigmoid, bias=battCol)
            pg = outpp.tile([128, H], f32, tag="outp")
            nc.tensor.matmul(pg, lhsT=scoreT, rhs=localSb, start=True, stop=True)
            nc.vector.tensor_copy(out=globSb, in_=pg)
        globDram = dramp.tile([N, H], f32, tag="globDram")
        nc.sync.dma_start(out=globDram, in_=globSb)
        split_x(globSb)
        load_g_rows(globDram)

        # ---------------- phase 2: global_pair ----------------
        out_phase(globSb, globDram, gp_d, with_attn=False)


def _build(reps=1):
    import concourse.bass as bass  # noqa: F401
    from concourse import bacc
    import concourse.mybir as mybir
    import concourse.tile as tile

    f32 = mybir.dt.float32
    nc = bacc.Bacc(
        "TRN2",
        target_bir_lowering=False,
        debug=False,
        enable_asserts=False,
        num_devices=NCORES,
    )
    io = (
        nc.dram_tensor("local", [N, H], f32, kind="ExternalInput").ap(),
        nc.dram_tensor("binary", [N, N, BIN], f32, kind="ExternalInput").ap(),
        nc.dram_tensor("w_apair", [H, H], f32, kind="ExternalInput").ap(),
        nc.dram_tensor("b_apair", [H], f32, kind="ExternalInput").ap(),
        nc.dram_tensor("w_binary", [BIN, H], f32, kind="ExternalInput").ap(),
        nc.dram_tensor("b_binary", [H], f32, kind="ExternalInput").ap(),
        nc.dram_tensor("w_att", [H, 1], f32, kind="ExternalInput").ap(),
        nc.dram_tensor("b_att", [1], f32, kind="ExternalInput").ap(),
        nc.dram_tensor("out_lp", [N, N, H], f32, kind="ExternalOutput").ap(),
        nc.dram_tensor("out_gp", [N, N, H], f32, kind="ExternalOutput").ap(),
    )
    with tile.TileContext(nc) as tc:
        _body(tc, io, reps=reps)
    nc.compile()
    return nc


def _get_nc():
    if "nc" not in _cache:
        _cache["nc"] = _build()
    return _cache["nc"]


def _run(inputs, trace=False):
    from concourse.bass_utils import run_bass_kernel_spmd

    nc = _get_nc()
    f = lambda x: np.ascontiguousarray(np.asarray(x), dtype=np.float32)
    shared = {
        "w_apair": f(inputs["W_apair"]),
        "b_apair": f(inputs["b_apair"]),
        "w_binary": f(inputs["W_binary"]),
        "b_binary": f(inputs["b_binary"]),
        "w_att": f(inputs["W_att"]),
        "b_att": f(inputs["b_att"]),
    }
    local = f(inputs["local_feats"])
    binary = f(inputs["binary_feats"])
    in_maps = [
        {"local": local[c], "binary": binary[c], **shared} for c in range(NCORES)
    ]
    res = run_bass_kernel_spmd(
        nc, in_maps, core_ids=list(range(NCORES)), trace=trace
    )
    lp = np.stack([r["out_lp"] for r in res.results])
    gp = np.stack([r["out_gp"] for r in res.results])
    return (lp, gp), res


def kernel(**inputs):
    out, _ = _run(inputs, trace=False)
    return out
